# revision 1
# baseline (speedup 1.0000x reference)
"""Trainium2 Bass kernel for nn_CrossAttentionBlock (B=8, C=256, H=W=48).

Sharding: data-parallel over batch B — one batch per NeuronCore (8 cores).

Per-core math (x: [C=256, N=2304] f32):
  LayerNorm over C is folded:
    - w_n / b_n folded into projection weights on host:
        W_eff = W * w_n[None,:],  b_eff = b + W @ b_n
    - attention SCALE folded into Wq_eff / bq_eff
    - per-location mean u[n] / rstd[n] computed on-device via a
      ones-matrix matmul (broadcasts u across all 128 partitions in one
      matmul), then xn = (x - u_b) * rstd_b on DVE.
  Attention is computed transposed:  St[m,n] = sum_o k[o,m] q[o,n]
  so softmax normalization runs over the *partition* axis m:
    - no row-max subtraction (logits bounded ~21, exp safe in f32)
    - P = exp(St) (ScalarE, PSUM->SBUF bf16 eviction)
    - rowsum[n] = sum_m P[m,n] via an M=1 ones-matmul, folded out as
      1/rowsum AFTER the output projection (scaling commutes with Wp).
  v is produced directly transposed (vT[m,o] = sum_c xn2[c,m] WvT[c,o])
  so P·V contracts over m on partitions with zero PE transposes.
"""

import os
import sys
import types
import ctypes
import contextlib

sys.path.insert(0, "/opt/trn_rl_repo")

import numpy as np
import ml_dtypes

# ---------------------------------------------------------------------------
# NTFF profile hook stub (antenv.axon_hooks is absent in this container; the
# ctypes shim mirrors trn_agent_boot). Only used when tracing is requested.
# ---------------------------------------------------------------------------


def _ntff_profile_via_ctypes(so_path):
    try:
        lib = ctypes.CDLL(so_path)
    except OSError:
        return None
    if not hasattr(lib, "axon_start_nrt_profile"):
        return None
    lib.axon_start_nrt_profile.argtypes = [
        ctypes.POINTER(ctypes.c_int64),
        ctypes.c_size_t,
    ]
    lib.axon_start_nrt_profile.restype = ctypes.c_int64
    lib.axon_stop_nrt_profile.argtypes = [ctypes.c_char_p]
    lib.axon_stop_nrt_profile.restype = ctypes.c_int64

    @contextlib.contextmanager
    def _hook(output_dir, device_ids):
        import jax

        jax.devices()
        if device_ids:
            ids = (ctypes.c_int64 * len(device_ids))(*device_ids)
            rc = lib.axon_start_nrt_profile(ids, len(device_ids))
        else:
            rc = lib.axon_start_nrt_profile(None, 0)
        if rc != 0:
            raise RuntimeError(f"axon_start_nrt_profile rc={rc}")
        try:
            yield
        finally:
            n = lib.axon_stop_nrt_profile(str(output_dir).encode())
            print(f"profile: {n} file(s) written to {output_dir}", file=sys.stderr)

    return _hook


if "antenv.axon_hooks" not in sys.modules:
    _hook = _ntff_profile_via_ctypes("/opt/axon/libaxon_pjrt.so")
    _mod = types.ModuleType("antenv.axon_hooks")
    _mod.get_axon_ntff_profile_hook = lambda: _hook
    sys.modules["antenv.axon_hooks"] = _mod

# ---------------------------------------------------------------------------

B, C, H, W = 8, 256, 48, 48
N = H * W  # 2304
SCALE = (C // 8) ** (-0.5)
EPS = 1e-6
CT = C // 128  # 2 channel tiles
MT = N // 128  # 18 m (key-token) tiles
CHUNKS = [(0, 512), (512, 512), (1024, 512), (1536, 512), (2048, 256)]

BF16 = ml_dtypes.bfloat16

_cache = {}
last_results = None  # BassKernelResults of the most recent run (for test.py)


def _build_program():
    import concourse.bacc as bacc
    import concourse.tile as tile
    import concourse.mybir as mybir
    from contextlib import ExitStack

    f32 = mybir.dt.float32
    bf16 = mybir.dt.bfloat16
    ADD = mybir.AluOpType.add
    SUB = mybir.AluOpType.subtract

    nc = bacc.Bacc("TRN2", target_bir_lowering=False, debug=False)

    x1_d = nc.dram_tensor("x1", [C, N], f32, kind="ExternalInput").ap()
    x2_d = nc.dram_tensor("x2", [C, N], f32, kind="ExternalInput").ap()
    wqt_d = nc.dram_tensor("wqt", [C, C], bf16, kind="ExternalInput").ap()
    wkt_d = nc.dram_tensor("wkt", [C, C], bf16, kind="ExternalInput").ap()
    wvt_d = nc.dram_tensor("wvt", [C, C], bf16, kind="ExternalInput").ap()
    wpt_d = nc.dram_tensor("wpt", [C, C], bf16, kind="ExternalInput").ap()
    # cvec columns: 0/1 = bq per o-tile, 2/3 = bk per o-tile, 4/5 = bp per
    # c-tile, 6:134 = 1.0 (f32 ones row used as K=1 lhsT for broadcasts).
    cvec_d = nc.dram_tensor("cvec", [128, 134], f32, kind="ExternalInput").ap()
    # cbf columns: 0:128 = 1/C (stats broadcast matmul), 128 = 1.0 (rowsum
    # lhsT), 132:260 = 1.0 (K=1 ones lhsT row on partition 0).
    cbf_d = nc.dram_tensor("cbf", [128, 260], bf16, kind="ExternalInput").ap()
    bvrow_d = nc.dram_tensor("bvrow", [1, C], bf16, kind="ExternalInput").ap()
    out_d = nc.dram_tensor("out", [C, N], f32, kind="ExternalOutput").ap()

    with tile.TileContext(nc) as tc, ExitStack() as ctx:
        persist = ctx.enter_context(tc.tile_pool(name="persist", bufs=1))

        # ---- constants -------------------------------------------------
        cvec = persist.tile([128, 134], f32, tag="cvec", name="cvec")
        nc.sync.dma_start(cvec[:], cvec_d[:, :])
        cbf = persist.tile([128, 260], bf16, tag="cbf", name="cbf")
        nc.sync.dma_start(cbf[:], cbf_d[:, :])
        bvrow = persist.tile([1, C], bf16, tag="bvrow", name="bvrow")
        nc.sync.dma_start(bvrow[:], bvrow_d[:, :])

        w_tiles = {}
        for nm, d in (("k", wkt_d), ("v", wvt_d), ("q", wqt_d), ("p", wpt_d)):
            for ct in range(CT):
                t = persist.tile([128, C], bf16, tag=f"w{nm}{ct}", name=f"w{nm}{ct}")
                nc.sync.dma_start(t[:], d[ct * 128 : (ct + 1) * 128, :])
                w_tiles[(nm, ct)] = t

        # persistent intermediates
        k_t = [persist.tile([128, N], bf16, tag=f"k{ot}", name=f"k{ot}") for ot in range(CT)]
        vT_t = [persist.tile([128, C], bf16, tag=f"vT{m}", name=f"vT{m}") for m in range(MT)]
        ou_t = [persist.tile([128, N], bf16, tag=f"ou{ct}", name=f"ou{ct}") for ct in range(CT)]
        rs_sb = persist.tile([1, N], f32, tag="rs", name="rs")
        inv_b = persist.tile([128, N], f32, tag="invb", name="invb")
        out_t = [persist.tile([128, N], f32, tag=f"out{ct}", name=f"out{ct}") for ct in range(CT)]
        x1_t = []
        for ct in range(CT):
            t = persist.tile([128, N], f32, tag=f"x1_{ct}", name=f"x1_{ct}")
            for off, w in CHUNKS:
                nc.sync.dma_start(
                    t[:, off : off + w], x1_d[ct * 128 : (ct + 1) * 128, off : off + w]
                )
            x1_t.append(t)

        with tc.tile_pool(name="mid1", bufs=1) as mid1:
            xn = {}
            with (
                tc.tile_pool(name="mid2", bufs=1) as mid2,
                tc.tile_pool(name="scr", bufs=2) as scr,
                tc.tile_pool(name="ps_st", bufs=2, space="PSUM") as ps_stats,
                tc.tile_pool(name="ps_pj", bufs=2, space="PSUM") as ps_pj,
            ):
                x2_t = []
                for ct in range(CT):
                    t = mid2.tile([128, N], f32, tag=f"x2_{ct}", name=f"x2_{ct}")
                    for off, w in CHUNKS:
                        nc.sync.dma_start(
                            t[:, off : off + w],
                            x2_d[ct * 128 : (ct + 1) * 128, off : off + w],
                        )
                    x2_t.append(t)

                # x2 pipeline first (k and vT gate all of attention); casts
                # for x2 on GpSimd, x1 on DVE so the two streams overlap.
                xb = {}
                for tsel, srct, eng in ((1, x2_t, nc.gpsimd), (0, x1_t, nc.vector)):
                    for ct in range(CT):
                        xb[(tsel, ct)] = mid2.tile(
                            [128, N], bf16, tag=f"xb{tsel}{ct}", name=f"xb{tsel}{ct}"
                        )
                        for off, w in CHUNKS:
                            eng.tensor_copy(
                                xb[(tsel, ct)][:, off : off + w],
                                srct[ct][:, off : off + w],
                            )

                # ---- stats + xn per (tensor, chunk) --------------------
                for tsel in (1, 0):
                    for ji, (off, w) in enumerate(CHUNKS):
                        ub = ps_stats.tile([128, w], f32, tag="ub", name="ub")
                        ms = ps_stats.tile([128, w], f32, tag="ms", name="ms")
                        for ct in range(CT):
                            nc.tensor.matmul(
                                ub[:],
                                cbf[:, 0:128],
                                xb[(tsel, ct)][:, off : off + w],
                                start=(ct == 0),
                                stop=(ct == CT - 1),
                            )
                        for ct in range(CT):
                            xsq_c = scr.tile([128, w], bf16, tag="xsqc", name="xsqc")
                            nc.gpsimd.tensor_mul(
                                xsq_c[:],
                                xb[(tsel, ct)][:, off : off + w],
                                xb[(tsel, ct)][:, off : off + w],
                            )
                            nc.tensor.matmul(
                                ms[:],
                                cbf[:, 0:128],
                                xsq_c[:],
                                start=(ct == 0),
                                stop=(ct == CT - 1),
                            )
                        usq = scr.tile([128, w], f32, tag="usq", name="usq")
                        nc.scalar.square(usq[:], ub[:])
                        var = scr.tile([128, w], f32, tag="var", name="var")
                        # var = (ms + eps) - u^2  (eps folded as an immediate)
                        nc.vector.scalar_tensor_tensor(var[:], ms[:], EPS, usq[:], ADD, SUB)
                        std = scr.tile([128, w], f32, tag="std", name="std")
                        nc.scalar.activation(std[:], var[:], mybir.ActivationFunctionType.Sqrt)
                        rstd = scr.tile([128, w], f32, tag="rstd", name="rstd")
                        nc.vector.reciprocal_approx_fast(rstd[:], std[:])
                        pool = mid2 if tsel == 1 else mid1
                        for ct in range(CT):
                            d = scr.tile([128, w], f32, tag="xnd", name="xnd")
                            nc.vector.tensor_sub(d[:], xb[(tsel, ct)][:, off : off + w], ub[:])
                            xt = pool.tile([128, w], bf16, tag=f"xn{tsel}{ct}{ji}", name=f"xn{tsel}{ct}{ji}")
                            nc.vector.tensor_mul(xt[:], d[:], rstd[:])
                            xn[(tsel, ct, ji)] = xt

                # ---- k projection -------------------------------------
                for ot in range(CT):
                    for ji, (off, w) in enumerate(CHUNKS):
                        ps = ps_pj.tile([128, 512], f32, tag="pj", name="pj")
                        for ct in range(CT):
                            nc.tensor.matmul(
                                ps[:, :w],
                                w_tiles[("k", ct)][:, ot * 128 : (ot + 1) * 128],
                                xn[(1, ct, ji)][:],
                                start=(ct == 0),
                                stop=(ct == CT - 1),
                            )
                        nc.vector.tensor_scalar_add(
                            k_t[ot][:, off : off + w], ps[:, :w], cvec[:, 2 + ot : 3 + ot]
                        )

                # ---- vT (v produced directly transposed) ---------------
                for m in range(MT):
                    col = m * 128
                    ji = min(col // 512, len(CHUNKS) - 1)
                    coff = col - CHUNKS[ji][0]
                    ps = ps_pj.tile([128, C], f32, tag="pv", name="pv")
                    for ct in range(CT):
                        nc.tensor.matmul(
                            ps[:],
                            xn[(1, ct, ji)][:, coff : coff + 128],
                            w_tiles[("v", ct)][:, :],
                            start=(ct == 0),
                            stop=False,
                        )
                    nc.tensor.matmul(
                        ps[:], cbf[0:1, 132:260], bvrow[0:1, :], start=False, stop=True
                    )
                    nc.vector.tensor_copy(vT_t[m][:], ps[:])

            # ---- attention: q projected per chunk, pipelined one ahead -
            with (
                tc.tile_pool(name="qch", bufs=2) as qch,
                tc.tile_pool(name="pt", bufs=2) as pt_pool,
                tc.tile_pool(name="ps_qp", bufs=2, space="PSUM") as ps_qp,
                tc.tile_pool(name="ps_qk", bufs=2, space="PSUM") as ps_qk,
                tc.tile_pool(name="ps_o", bufs=2, space="PSUM") as ps_o,
                tc.tile_pool(name="ps_rs", bufs=2, space="PSUM") as ps_rs,
            ):
                q_ch = {}

                def emit_qproj(ji):
                    off, w = CHUNKS[ji]
                    for ot in range(CT):
                        ps = ps_qp.tile([128, 512], f32, tag="qp", name="qp")
                        for ct in range(CT):
                            nc.tensor.matmul(
                                ps[:, :w],
                                w_tiles[("q", ct)][:, ot * 128 : (ot + 1) * 128],
                                xn[(0, ct, ji)][:],
                                start=(ct == 0),
                                stop=(ct == CT - 1),
                            )
                        qt = qch.tile([128, 512], bf16, tag=f"q{ot}", name=f"q{ot}")
                        nc.vector.tensor_scalar_add(
                            qt[:, :w], ps[:, :w], cvec[:, 0 + ot : 1 + ot]
                        )
                        q_ch[(ji, ot)] = qt

                emit_qproj(0)
                for ji, (off, w) in enumerate(CHUNKS):
                    if ji + 1 < len(CHUNKS):
                        emit_qproj(ji + 1)
                    st = {}

                    def emit_qk(m):
                        ps = ps_qk.tile([128, 512], f32, tag="st", name="st")
                        for ot in range(CT):
                            nc.tensor.matmul(
                                ps[:, :w],
                                k_t[ot][:, m * 128 : (m + 1) * 128],
                                q_ch[(ji, ot)][:, :w],
                                start=(ot == 0),
                                stop=(ot == CT - 1),
                            )
                        st[m] = ps

                    o_ps = [ps_o.tile([128, 512], f32, tag="o", name="o") for _ in range(CT)]
                    rs_ps = ps_rs.tile([1, 512], f32, tag="rsp", name="rsp")

                    emit_qk(0)
                    for m in range(MT):
                        if m + 1 < MT:
                            emit_qk(m + 1)
                        pt = pt_pool.tile([128, w], bf16, tag=f"pt{m}", name=f"pt{m}")
                        nc.scalar.activation(
                            pt[:], st[m][:, :w], mybir.ActivationFunctionType.Exp
                        )
                        del st[m]
                        for c in range(CT):
                            nc.tensor.matmul(
                                o_ps[c][:, :w],
                                vT_t[m][:, c * 128 : (c + 1) * 128],
                                pt[:],
                                start=(m == 0),
                                stop=(m == MT - 1),
                            )
                        nc.tensor.matmul(
                            rs_ps[:, :w],
                            cbf[:, 128:129],
                            pt[:],
                            start=(m == 0),
                            stop=(m == MT - 1),
                        )
                    for c in range(CT):
                        nc.vector.tensor_copy(ou_t[c][:, off : off + w], o_ps[c][:, :w])
                    nc.vector.tensor_copy(rs_sb[0:1, off : off + w], rs_ps[0:1, :w])

        # ---- 1/rowsum broadcast, Wp projection, residual ---------------
        with (
            tc.tile_pool(name="fscr", bufs=4) as fscr,
            tc.tile_pool(name="ps_bc", bufs=2, space="PSUM") as ps_bc,
            tc.tile_pool(name="ps_p", bufs=4, space="PSUM") as ps_p,
        ):
            for ji, (off, w) in enumerate(CHUNKS):
                bc = ps_bc.tile([128, 512], f32, tag="bc", name="bc")
                nc.tensor.matmul(
                    bc[:, :w], cvec[0:1, 6:134], rs_sb[0:1, off : off + w],
                    start=True, stop=True,
                )
                nc.vector.reciprocal_approx_fast(inv_b[:, off : off + w], bc[:, :w])

            for ct in range(CT):
                for ji, (off, w) in enumerate(CHUNKS):
                    ps = ps_p.tile([128, 512], f32, tag="pp", name="pp")
                    for ci in range(CT):
                        nc.tensor.matmul(
                            ps[:, :w],
                            w_tiles[("p", ci)][:, ct * 128 : (ct + 1) * 128],
                            ou_t[ci][:, off : off + w],
                            start=(ci == 0),
                            stop=(ci == CT - 1),
                        )
                    sc = fscr.tile([128, 512], f32, tag="fs", name="fs")
                    nc.vector.tensor_mul(sc[:, :w], ps[:, :w], inv_b[:, off : off + w])
                    nc.vector.scalar_tensor_tensor(
                        out_t[ct][:, off : off + w],
                        sc[:, :w],
                        cvec[:, 4 + ct : 5 + ct],
                        x1_t[ct][:, off : off + w],
                        ADD,
                        ADD,
                    )
                nc.sync.dma_start(out_d[ct * 128 : (ct + 1) * 128, :], out_t[ct][:])

    nc.compile()
    return nc


def _host_prep(inputs):
    f = lambda k: np.asarray(inputs[k], dtype=np.float32)
    Wq, Wk, Wv, Wp = f("Wq"), f("Wk"), f("Wv"), f("Wp")
    bq, bk, bv, bp = f("bq"), f("bk"), f("bv"), f("bp")
    w_nq, b_nq, w_nkv, b_nkv = f("w_nq"), f("b_nq"), f("w_nkv"), f("b_nkv")

    Wq_eff = Wq * w_nq[None, :] * SCALE
    bq_eff = SCALE * (bq + Wq @ b_nq)
    Wk_eff = Wk * w_nkv[None, :]
    bk_eff = bk + Wk @ b_nkv
    Wv_eff = Wv * w_nkv[None, :]
    bv_eff = bv + Wv @ b_nkv

    wqt = np.ascontiguousarray(Wq_eff.T).astype(BF16)
    wkt = np.ascontiguousarray(Wk_eff.T).astype(BF16)
    wvt = np.ascontiguousarray(Wv_eff.T).astype(BF16)
    wpt = np.ascontiguousarray(Wp.T).astype(BF16)

    cvec = np.zeros((128, 134), np.float32)
    cvec[:, 0] = bq_eff[0:128]
    cvec[:, 1] = bq_eff[128:256]
    cvec[:, 2] = bk_eff[0:128]
    cvec[:, 3] = bk_eff[128:256]
    cvec[:, 4] = bp[0:128]
    cvec[:, 5] = bp[128:256]
    cvec[:, 6:134] = 1.0

    cbf = np.zeros((128, 260), np.float32)
    cbf[:, 0:128] = 1.0 / C
    cbf[:, 128] = 1.0
    cbf[:, 132:260] = 1.0
    cbf = cbf.astype(BF16)

    bvrow = bv_eff.reshape(1, C).astype(BF16)
    return dict(wqt=wqt, wkt=wkt, wvt=wvt, wpt=wpt, cvec=cvec, cbf=cbf, bvrow=bvrow)


def _maybe_patch_ldw_opt():
    if os.environ.get("BASS_LDW_OPT", "0") != "1":
        return
    import concourse.bass_utils as bu
    if getattr(bu, "_ldw_patch", False):
        return
    orig = bu.run_command
    def patched(argv, **kw):
        if isinstance(argv, list):
            argv = [a.replace("--enable-ldw-opt=false", "--enable-ldw-opt=true") for a in argv]
        return orig(argv, **kw)
    bu.run_command = patched
    bu._ldw_patch = True


def kernel(**inputs):
    global last_results
    _maybe_patch_ldw_opt()
    from concourse.bass_utils import run_bass_kernel_spmd

    if "nc" not in _cache:
        _cache["nc"] = _build_program()
    nc = _cache["nc"]

    shared = _host_prep(inputs)
    x1 = np.asarray(inputs["x1"], dtype=np.float32).reshape(B, C, N)
    x2 = np.asarray(inputs["x2"], dtype=np.float32).reshape(B, C, N)

    in_maps = []
    for b in range(B):
        m = dict(shared)
        m["x1"] = np.ascontiguousarray(x1[b])
        m["x2"] = np.ascontiguousarray(x2[b])
        in_maps.append(m)

    trace = os.environ.get("BASS_KERNEL_TRACE", "0") == "1"
    res = run_bass_kernel_spmd(
        nc, in_maps, core_ids=list(range(B)), trace=trace
    )
    last_results = res
    out = np.stack([res.results[b]["out"].reshape(C, H, W) for b in range(B)])
    return out.astype(np.float32)



# revision 9
# speedup vs baseline: 1.0713x; 1.0713x over previous
"""Trainium2 Bass kernel for nn_CrossAttentionBlock (B=8, C=256, H=W=48).

Sharding: data-parallel over batch B — one batch per NeuronCore (8 cores).

Per-core math (x: [C=256, N=2304] f32):
  LayerNorm over C folded into projection weights on host:
      W_eff = W * w_n[None,:],  b_eff = b + W @ b_n
  attention SCALE folded into Wq_eff / bq_eff.
  v bias folded into the output bias (softmax rows sum to one, so
  attn@(v+bv) = attn@v + bv, hence bp_eff = bp + Wp@bv_eff).

  Stats: mean via a 1/C-ones bf16 matmul on a bf16 cast of x (casts on
  Scalar for x2, Pool for x1 — the engines that are otherwise idle in the
  pre-phase); mean-square via a Pool-computed x*x (f32 in, bf16 out) fed
  to a second ones-matmul.  xn = (x-u)*rstd is computed from the raw f32
  x by the DVE, emitted as bf16.

  Attention is computed transposed:  St[m,n] = sum_o k[o,m] q[o,n]
  so softmax normalization runs over the *partition* axis m:
    - no row-max subtraction (logits bounded ~21, exp safe in f32)
    - P = exp(St) (ScalarE, PSUM->SBUF bf16 eviction)
    - rowsum[n] = sum_m P[m,n] via an M=1 ones-matmul accumulated across m
    - 1/rowsum applied AFTER the output projection (scaling commutes
      with Wp); the reciprocal row is partition-broadcast by the Pool
      engine and fused into the PSUM->SBUF eviction of the attention
      output, and the Wp projection + residual + DMA-out run per chunk
      inside the attention loop so there is no serial tail.
  v is produced directly transposed (vT[m,o] = sum_c xn2[c,m] WvT[c,o])
  so P.V contracts over m on partitions with zero PE transposes.
"""

import os
import sys
import types
import ctypes
import contextlib

sys.path.insert(0, "/opt/trn_rl_repo")

import numpy as np
import ml_dtypes

# ---------------------------------------------------------------------------
# NTFF profile hook stub (antenv.axon_hooks is absent in this container; the
# ctypes shim mirrors trn_agent_boot). Only used when tracing is requested.
# ---------------------------------------------------------------------------


def _ntff_profile_via_ctypes(so_path):
    try:
        lib = ctypes.CDLL(so_path)
    except OSError:
        return None
    if not hasattr(lib, "axon_start_nrt_profile"):
        return None
    lib.axon_start_nrt_profile.argtypes = [
        ctypes.POINTER(ctypes.c_int64),
        ctypes.c_size_t,
    ]
    lib.axon_start_nrt_profile.restype = ctypes.c_int64
    lib.axon_stop_nrt_profile.argtypes = [ctypes.c_char_p]
    lib.axon_stop_nrt_profile.restype = ctypes.c_int64

    @contextlib.contextmanager
    def _hook(output_dir, device_ids):
        import jax

        jax.devices()
        if device_ids:
            ids = (ctypes.c_int64 * len(device_ids))(*device_ids)
            rc = lib.axon_start_nrt_profile(ids, len(device_ids))
        else:
            rc = lib.axon_start_nrt_profile(None, 0)
        if rc != 0:
            raise RuntimeError(f"axon_start_nrt_profile rc={rc}")
        try:
            yield
        finally:
            n = lib.axon_stop_nrt_profile(str(output_dir).encode())
            print(f"profile: {n} file(s) written to {output_dir}", file=sys.stderr)

    return _hook


if "antenv.axon_hooks" not in sys.modules:
    _hook = _ntff_profile_via_ctypes("/opt/axon/libaxon_pjrt.so")
    _mod = types.ModuleType("antenv.axon_hooks")
    _mod.get_axon_ntff_profile_hook = lambda: _hook
    sys.modules["antenv.axon_hooks"] = _mod

# ---------------------------------------------------------------------------

B, C, H, W = 8, 256, 48, 48
N = H * W  # 2304
SCALE = (C // 8) ** (-0.5)
EPS = 1e-6
CT = C // 128  # 2 channel tiles
MT = N // 128  # 18 m (key-token) tiles
CHUNKS = [(0, 512), (512, 512), (1024, 512), (1536, 512), (2048, 256)]
NJ = len(CHUNKS)

BF16 = ml_dtypes.bfloat16

_cache = {}
last_results = None  # BassKernelResults of the most recent run (for test.py)


def _build_program():
    import concourse.bacc as bacc
    import concourse.tile as tile
    import concourse.mybir as mybir
    from contextlib import ExitStack

    f32 = mybir.dt.float32
    bf16 = mybir.dt.bfloat16
    ADD = mybir.AluOpType.add
    SUB = mybir.AluOpType.subtract

    nc = bacc.Bacc("TRN2", target_bir_lowering=False, debug=False)

    x1_d = nc.dram_tensor("x1", [C, N], f32, kind="ExternalInput").ap()
    x2_d = nc.dram_tensor("x2", [C, N], f32, kind="ExternalInput").ap()
    wqt_d = nc.dram_tensor("wqt", [C, C], bf16, kind="ExternalInput").ap()
    wkt_d = nc.dram_tensor("wkt", [C, C], bf16, kind="ExternalInput").ap()
    wvt_d = nc.dram_tensor("wvt", [C, C], bf16, kind="ExternalInput").ap()
    wpt_d = nc.dram_tensor("wpt", [C, C], bf16, kind="ExternalInput").ap()
    # cvec columns: 0/1 = bq per o-tile, 2/3 = bk per o-tile, 4/5 = bp_eff per
    # c-tile.
    cvec_d = nc.dram_tensor("cvec", [128, 6], f32, kind="ExternalInput").ap()
    # cbf columns: 0:128 = 1/C bf16 (mean-square lhsT), 128 = 1.0 (rowsum lhsT).
    cbf_d = nc.dram_tensor("cbf", [128, 129], bf16, kind="ExternalInput").ap()
    out_d = nc.dram_tensor("out", [C, N], f32, kind="ExternalOutput").ap()

    # m-tiles covered by each chunk: chunk j covers m in [off/128, (off+w)/128)
    def chunk_mtiles(ji):
        off, w = CHUNKS[ji]
        return range(off // 128, (off + w) // 128)

    with tile.TileContext(nc) as tc, ExitStack() as ctx:
        persist = ctx.enter_context(tc.tile_pool(name="persist", bufs=1))

        # ---- constants (DMA'd first: small) ----------------------------
        cvec = persist.tile([128, 6], f32, tag="cvec", name="cvec")
        nc.sync.dma_start(cvec[:], cvec_d[:, :])
        cbf = persist.tile([128, 129], bf16, tag="cbf", name="cbf")
        nc.sync.dma_start(cbf[:], cbf_d[:, :])

        w_tiles = {}
        for nm, d in (("k", wkt_d), ("v", wvt_d), ("q", wqt_d), ("p", wpt_d)):
            for ct in range(CT):
                t = persist.tile([128, C], bf16, tag=f"w{nm}{ct}", name=f"w{nm}{ct}")
                nc.sync.dma_start(t[:], d[ct * 128 : (ct + 1) * 128, :])
                w_tiles[(nm, ct)] = t

        # ---- activation DMA: x2 first (it gates k/v -> attention), -----
        # ---- chunk-major, sub-split for queue parallelism --------------
        x2sc = ctx.enter_context(tc.tile_pool(name="x2scope", bufs=1))
        x2_t = [
            x2sc.tile([128, N], f32, tag=f"x2_{ct}", name=f"x2_{ct}")
            for ct in range(CT)
        ]
        for off, w in CHUNKS:
            for ct in range(CT):
                for s in range(off, off + w, 256):
                    sw = min(256, off + w - s)
                    nc.sync.dma_start(
                        x2_t[ct][:, s : s + sw],
                        x2_d[ct * 128 : (ct + 1) * 128, s : s + sw],
                    )
        x1_t = [
            persist.tile([128, N], f32, tag=f"x1_{ct}", name=f"x1_{ct}")
            for ct in range(CT)
        ]
        for off, w in CHUNKS:
            for ct in range(CT):
                for s in range(off, off + w, 256):
                    sw = min(256, off + w - s)
                    nc.sync.dma_start(
                        x1_t[ct][:, s : s + sw],
                        x1_d[ct * 128 : (ct + 1) * 128, s : s + sw],
                    )

        # persistent intermediates
        k_t = [
            persist.tile([128, N], bf16, tag=f"k{ot}", name=f"k{ot}")
            for ot in range(CT)
        ]
        vT_t = [
            persist.tile([128, C], bf16, tag=f"vT{m}", name=f"vT{m}")
            for m in range(MT)
        ]
        xn1_t = [
            persist.tile([128, N], bf16, tag=f"xn1_{ct}", name=f"xn1_{ct}")
            for ct in range(CT)
        ]

        # ------------------------------------------------------------------
        # Pre-phase: per-chunk pipeline  stats -> xn -> k/vT   (x2 stream)
        # plus the x1 stats/xn stream (feeds q projections later).
        # ------------------------------------------------------------------
        with (
            tc.tile_pool(name="scr", bufs=3) as scr,
            tc.tile_pool(name="xnp", bufs=6) as xnp,
            tc.tile_pool(name="ps_st", bufs=2, space="PSUM") as ps_st,
            tc.tile_pool(name="ps_kv", bufs=2, space="PSUM") as ps_kv,
        ):

            def emit_stats_xn(tsel, ji, xsrc, xn_out):
                """stats + xn for (tensor tsel, chunk ji).

                xsrc: list of [128, N] f32 tiles (per ct)
                xn_out: dict key (ct) -> (tile, col_off) destination slices
                """
                off, w = CHUNKS[ji]
                ub = ps_st.tile([128, 512], f32, tag="ub", name="ub")
                for ct in range(CT):
                    xb = scr.tile([128, 512], bf16, tag="xb", name="xb")
                    if tsel == 1:
                        nc.scalar.copy(xb[:, :w], xsrc[ct][:, off : off + w])
                    else:
                        nc.gpsimd.tensor_copy(xb[:, :w], xsrc[ct][:, off : off + w])
                    nc.tensor.matmul(
                        ub[:, :w],
                        cbf[:, 0:128],
                        xb[:, :w],
                        start=(ct == 0),
                        stop=(ct == CT - 1),
                    )
                ms = ps_st.tile([128, 512], f32, tag="ms", name="ms")
                for ct in range(CT):
                    xsq = scr.tile([128, 512], bf16, tag="xsq", name="xsq")
                    nc.gpsimd.tensor_mul(
                        xsq[:, :w],
                        xsrc[ct][:, off : off + w],
                        xsrc[ct][:, off : off + w],
                    )
                    nc.tensor.matmul(
                        ms[:, :w],
                        cbf[:, 0:128],
                        xsq[:, :w],
                        start=(ct == 0),
                        stop=(ct == CT - 1),
                    )
                usq = scr.tile([128, 512], f32, tag="usq", name="usq")
                nc.scalar.square(usq[:, :w], ub[:, :w])
                var = scr.tile([128, 512], f32, tag="var", name="var")
                nc.vector.scalar_tensor_tensor(
                    var[:, :w], ms[:, :w], EPS, usq[:, :w], ADD, SUB
                )
                std = scr.tile([128, 512], f32, tag="std", name="std")
                nc.scalar.activation(
                    std[:, :w], var[:, :w], mybir.ActivationFunctionType.Sqrt
                )
                rstd = scr.tile([128, 512], f32, tag=f"rstd{tsel}", name=f"rstd{tsel}")
                nc.vector.reciprocal_approx_fast(rstd[:, :w], std[:, :w])
                for ct in range(CT):
                    d = scr.tile([128, 512], f32, tag="xnd", name="xnd")
                    nc.vector.tensor_sub(
                        d[:, :w], xsrc[ct][:, off : off + w], ub[:, :w]
                    )
                    dst, dcol = xn_out[ct]
                    nc.vector.tensor_mul(
                        dst[:, dcol : dcol + w], d[:, :w], rstd[:, :w]
                    )

            xn2 = {}

            def emit_kv(ji):
                off, w = CHUNKS[ji]
                # k projection for this chunk of tokens
                for ot in range(CT):
                    ps = ps_kv.tile([128, 512], f32, tag="kv", name="kv")
                    for ct in range(CT):
                        nc.tensor.matmul(
                            ps[:, :w],
                            w_tiles[("k", ct)][:, ot * 128 : (ot + 1) * 128],
                            xn2[(ji, ct)][:, :w],
                            start=(ct == 0),
                            stop=(ct == CT - 1),
                        )
                    nc.vector.tensor_scalar_add(
                        k_t[ot][:, off : off + w], ps[:, :w], cvec[:, 2 + ot : 3 + ot]
                    )
                # vT for the m-tiles inside this chunk
                for m in chunk_mtiles(ji):
                    coff = m * 128 - off
                    ps = ps_kv.tile([128, C], f32, tag="kv", name="kv")
                    for ct in range(CT):
                        nc.tensor.matmul(
                            ps[:],
                            xn2[(ji, ct)][:, coff : coff + 128],
                            w_tiles[("v", ct)][:, :],
                            start=(ct == 0),
                            stop=(ct == CT - 1),
                        )
                    nc.scalar.copy(vT_t[m][:], ps[:])

            # x2 chunk 0 first (longest chain), then interleave x1 stats so
            # the PE always has stats matmuls to chew on while DVE chains run.
            for ji in range(NJ):
                for ct in range(CT):
                    t = xnp.tile([128, 512], bf16, tag="xn2", name=f"xn2_{ji}_{ct}")
                    xn2[(ji, ct)] = t
                emit_stats_xn(1, ji, x2_t, {ct: (xn2[(ji, ct)], 0) for ct in range(CT)})
                emit_kv(ji)
                # x1 stream trails: its xn goes to the persistent xn1 tiles
                emit_stats_xn(
                    0, ji, x1_t, {ct: (xn1_t[ct], CHUNKS[ji][0]) for ct in range(CT)}
                )

        # ------------------------------------------------------------------
        # Attention: per q-chunk; q projected one chunk ahead; epilogue
        # (normalize, Wp projection, residual, DMA out) inside the loop.
        # ------------------------------------------------------------------
        with (
            tc.tile_pool(name="qch", bufs=4) as qch,
            tc.tile_pool(name="pt", bufs=4) as pt_pool,
            tc.tile_pool(name="oup", bufs=4) as oup,
            tc.tile_pool(name="invp", bufs=2) as invp,
            tc.tile_pool(name="outp", bufs=4) as outp,
            tc.tile_pool(name="ps_pj", bufs=2, space="PSUM") as ps_pj,
            tc.tile_pool(name="ps_qk", bufs=2, space="PSUM") as ps_qk,
            tc.tile_pool(name="ps_o", bufs=2, space="PSUM") as ps_o,
            tc.tile_pool(name="ps_rs", bufs=1, space="PSUM") as ps_rs,
        ):
            q_ch = {}

            def emit_qproj(ji):
                off, w = CHUNKS[ji]
                for ot in range(CT):
                    ps = ps_pj.tile([128, 512], f32, tag="pj", name="pj")
                    for ct in range(CT):
                        nc.tensor.matmul(
                            ps[:, :w],
                            w_tiles[("q", ct)][:, ot * 128 : (ot + 1) * 128],
                            xn1_t[ct][:, off : off + w],
                            start=(ct == 0),
                            stop=(ct == CT - 1),
                        )
                    qt = qch.tile([128, 512], bf16, tag="q", name=f"q{ji}_{ot}")
                    nc.vector.tensor_scalar_add(
                        qt[:, :w], ps[:, :w], cvec[:, 0 + ot : 1 + ot]
                    )
                    q_ch[(ji, ot)] = qt

            emit_qproj(0)
            for ji, (off, w) in enumerate(CHUNKS):
                if ji + 1 < NJ:
                    emit_qproj(ji + 1)
                st = {}

                def emit_qk(m):
                    ps = ps_qk.tile([128, 512], f32, tag="st", name="st")
                    for ot in range(CT):
                        nc.tensor.matmul(
                            ps[:, :w],
                            k_t[ot][:, m * 128 : (m + 1) * 128],
                            q_ch[(ji, ot)][:, :w],
                            start=(ot == 0),
                            stop=(ot == CT - 1),
                        )
                    st[m] = ps

                o_ps = [
                    ps_o.tile([128, 512], f32, tag="o", name="o") for _ in range(CT)
                ]
                rs_ps = ps_rs.tile([1, 512], f32, tag="rsp", name="rsp")

                emit_qk(0)
                for m in range(MT):
                    if m + 1 < MT:
                        emit_qk(m + 1)
                    pt = pt_pool.tile([128, 512], bf16, tag="pt", name=f"pt{m}")
                    nc.scalar.activation(
                        pt[:, :w], st[m][:, :w], mybir.ActivationFunctionType.Exp
                    )
                    del st[m]
                    for c in range(CT):
                        nc.tensor.matmul(
                            o_ps[c][:, :w],
                            vT_t[m][:, c * 128 : (c + 1) * 128],
                            pt[:, :w],
                            start=(m == 0),
                            stop=(m == MT - 1),
                        )
                    nc.tensor.matmul(
                        rs_ps[:, :w],
                        cbf[:, 128:129],
                        pt[:, :w],
                        start=(m == 0),
                        stop=(m == MT - 1),
                    )

                # ---- chunk epilogue -----------------------------------
                invrow = invp.tile([1, 512], f32, tag="invr", name="invr")
                nc.vector.reciprocal_approx_fast(invrow[0:1, :w], rs_ps[0:1, :w])
                inv_b = invp.tile([128, 512], f32, tag="invb", name="invb")
                nc.gpsimd.partition_broadcast(inv_b[:, :w], invrow[0:1, :w])

                ou = []
                for c in range(CT):
                    t = oup.tile([128, 512], bf16, tag="ou", name=f"ou{c}")
                    nc.vector.tensor_mul(t[:, :w], o_ps[c][:, :w], inv_b[:, :w])
                    ou.append(t)

                for ct in range(CT):
                    ps = ps_pj.tile([128, 512], f32, tag="pj", name="pj")
                    for ci in range(CT):
                        nc.tensor.matmul(
                            ps[:, :w],
                            w_tiles[("p", ci)][:, ct * 128 : (ct + 1) * 128],
                            ou[ci][:, :w],
                            start=(ci == 0),
                            stop=(ci == CT - 1),
                        )
                    ot_t = outp.tile([128, 512], f32, tag="outt", name=f"out{ct}")
                    nc.vector.scalar_tensor_tensor(
                        ot_t[:, :w],
                        ps[:, :w],
                        cvec[:, 4 + ct : 5 + ct],
                        x1_t[ct][:, off : off + w],
                        ADD,
                        ADD,
                    )
                    nc.sync.dma_start(
                        out_d[ct * 128 : (ct + 1) * 128, off : off + w], ot_t[:, :w]
                    )

    nc.compile()
    return nc


def _host_prep(inputs):
    f = lambda k: np.asarray(inputs[k], dtype=np.float32)
    Wq, Wk, Wv, Wp = f("Wq"), f("Wk"), f("Wv"), f("Wp")
    bq, bk, bv, bp = f("bq"), f("bk"), f("bv"), f("bp")
    w_nq, b_nq, w_nkv, b_nkv = f("w_nq"), f("b_nq"), f("w_nkv"), f("b_nkv")

    Wq_eff = Wq * w_nq[None, :] * SCALE
    bq_eff = SCALE * (bq + Wq @ b_nq)
    Wk_eff = Wk * w_nkv[None, :]
    bk_eff = bk + Wk @ b_nkv
    Wv_eff = Wv * w_nkv[None, :]
    bv_eff = bv + Wv @ b_nkv
    bp_eff = bp + Wp @ bv_eff  # v bias folded through softmax + Wp

    wqt = np.ascontiguousarray(Wq_eff.T).astype(BF16)
    wkt = np.ascontiguousarray(Wk_eff.T).astype(BF16)
    wvt = np.ascontiguousarray(Wv_eff.T).astype(BF16)
    wpt = np.ascontiguousarray(Wp.T).astype(BF16)

    cvec = np.zeros((128, 6), np.float32)
    cvec[:, 0] = bq_eff[0:128]
    cvec[:, 1] = bq_eff[128:256]
    cvec[:, 2] = bk_eff[0:128]
    cvec[:, 3] = bk_eff[128:256]
    cvec[:, 4] = bp_eff[0:128]
    cvec[:, 5] = bp_eff[128:256]

    cbf = np.zeros((128, 129), np.float32)
    cbf[:, 0:128] = 1.0 / C
    cbf[:, 128] = 1.0
    cbf = cbf.astype(BF16)

    return dict(wqt=wqt, wkt=wkt, wvt=wvt, wpt=wpt, cvec=cvec, cbf=cbf)


def _maybe_patch_ldw_opt():
    if os.environ.get("BASS_LDW_OPT", "0") != "1":
        return
    import concourse.bass_utils as bu
    if getattr(bu, "_ldw_patch", False):
        return
    orig = bu.run_command
    def patched(argv, **kw):
        if isinstance(argv, list):
            argv = [a.replace("--enable-ldw-opt=false", "--enable-ldw-opt=true") for a in argv]
        return orig(argv, **kw)
    bu.run_command = patched
    bu._ldw_patch = True


def kernel(**inputs):
    global last_results
    _maybe_patch_ldw_opt()
    from concourse.bass_utils import run_bass_kernel_spmd

    if "nc" not in _cache:
        _cache["nc"] = _build_program()
    nc = _cache["nc"]

    shared = _host_prep(inputs)
    x1 = np.asarray(inputs["x1"], dtype=np.float32).reshape(B, C, N)
    x2 = np.asarray(inputs["x2"], dtype=np.float32).reshape(B, C, N)

    in_maps = []
    for b in range(B):
        m = dict(shared)
        m["x1"] = np.ascontiguousarray(x1[b])
        m["x2"] = np.ascontiguousarray(x2[b])
        in_maps.append(m)

    trace = os.environ.get("BASS_KERNEL_TRACE", "0") == "1"
    res = run_bass_kernel_spmd(
        nc, in_maps, core_ids=list(range(B)), trace=trace
    )
    last_results = res
    out = np.stack([res.results[b]["out"].reshape(C, H, W) for b in range(B)])
    return out.astype(np.float32)


# revision 18
# speedup vs baseline: 1.1151x; 1.0409x over previous
"""Trainium2 Bass kernel for nn_CrossAttentionBlock (B=8, C=256, H=W=48).

Sharding: data-parallel over batch B — one batch per NeuronCore (8 cores).

Per-core math (x: [C=256, N=2304] f32):
  LayerNorm over C folded into projection weights on host:
      W_eff = W * w_n[None,:],  b_eff = b + W @ b_n
  attention SCALE folded into Wq_eff / bq_eff.
  v bias folded into the output bias (softmax rows sum to one, so
  attn@(v+bv) = attn@v + bv, hence bp_eff = bp + Wp@bv_eff).

  Activations are host-cast to bf16 and DMA'd as [32, N] partition strips
  (the DMA queues are descriptor-rate-bound; full-row descriptors maximize
  bytes per descriptor).  Stats: mean via a 1/C-ones bf16 matmul on x;
  mean-square via a Pool-computed x*x fed to a second ones-matmul.
  xn = (x-u)*rstd on the DVE, emitted as bf16.

  Attention is computed transposed:  St[m,n] = sum_o k[o,m] q[o,n]
  so softmax normalization runs over the *partition* axis m:
    - no row-max subtraction (logits bounded ~21, exp safe in f32)
    - P = exp(St) (ScalarE, PSUM->SBUF bf16 eviction)
    - rowsum[n] = sum_m P[m,n] via an M=1 ones-matmul accumulated across m
    - 1/rowsum applied AFTER the output projection (scaling commutes
      with Wp); the reciprocal row is partition-broadcast by the Pool
      engine and fused into the PSUM->SBUF eviction of the attention
      output, and the Wp projection + residual + DMA-out run per chunk
      inside the attention loop so there is no serial tail.
  v is produced directly transposed (vT[m,o] = sum_c xn2[c,m] WvT[c,o])
  so P.V contracts over m on partitions with zero PE transposes.
"""

import os
import sys
import types
import ctypes
import contextlib

sys.path.insert(0, "/opt/trn_rl_repo")

import numpy as np
import ml_dtypes

# ---------------------------------------------------------------------------
# NTFF profile hook stub (antenv.axon_hooks is absent in this container; the
# ctypes shim mirrors trn_agent_boot). Only used when tracing is requested.
# ---------------------------------------------------------------------------


def _ntff_profile_via_ctypes(so_path):
    try:
        lib = ctypes.CDLL(so_path)
    except OSError:
        return None
    if not hasattr(lib, "axon_start_nrt_profile"):
        return None
    lib.axon_start_nrt_profile.argtypes = [
        ctypes.POINTER(ctypes.c_int64),
        ctypes.c_size_t,
    ]
    lib.axon_start_nrt_profile.restype = ctypes.c_int64
    lib.axon_stop_nrt_profile.argtypes = [ctypes.c_char_p]
    lib.axon_stop_nrt_profile.restype = ctypes.c_int64

    @contextlib.contextmanager
    def _hook(output_dir, device_ids):
        import jax

        jax.devices()
        if device_ids:
            ids = (ctypes.c_int64 * len(device_ids))(*device_ids)
            rc = lib.axon_start_nrt_profile(ids, len(device_ids))
        else:
            rc = lib.axon_start_nrt_profile(None, 0)
        if rc != 0:
            raise RuntimeError(f"axon_start_nrt_profile rc={rc}")
        try:
            yield
        finally:
            n = lib.axon_stop_nrt_profile(str(output_dir).encode())
            print(f"profile: {n} file(s) written to {output_dir}", file=sys.stderr)

    return _hook


if "antenv.axon_hooks" not in sys.modules:
    _hook = _ntff_profile_via_ctypes("/opt/axon/libaxon_pjrt.so")
    _mod = types.ModuleType("antenv.axon_hooks")
    _mod.get_axon_ntff_profile_hook = lambda: _hook
    sys.modules["antenv.axon_hooks"] = _mod

# ---------------------------------------------------------------------------

B, C, H, W = 8, 256, 48, 48
N = H * W  # 2304
SCALE = (C // 8) ** (-0.5)
EPS = 1e-6
CT = C // 128  # 2 channel tiles
MT = N // 128  # 18 m (key-token) tiles
CHUNKS = [(0, 512), (512, 512), (1024, 512), (1536, 512), (2048, 256)]
NJ = len(CHUNKS)

BF16 = ml_dtypes.bfloat16

_cache = {}
last_results = None  # BassKernelResults of the most recent run (for test.py)


def _build_program():
    import concourse.bacc as bacc
    import concourse.tile as tile
    import concourse.mybir as mybir
    from contextlib import ExitStack

    f32 = mybir.dt.float32
    bf16 = mybir.dt.bfloat16
    ADD = mybir.AluOpType.add
    SUB = mybir.AluOpType.subtract

    nc = bacc.Bacc("TRN2", target_bir_lowering=False, debug=False)

    x1_d = nc.dram_tensor("x1", [C, N], bf16, kind="ExternalInput").ap()
    x2_d = nc.dram_tensor("x2", [C, N], bf16, kind="ExternalInput").ap()
    wqt_d = nc.dram_tensor("wqt", [C, C], bf16, kind="ExternalInput").ap()
    wkt_d = nc.dram_tensor("wkt", [C, C], bf16, kind="ExternalInput").ap()
    wvt_d = nc.dram_tensor("wvt", [C, C], bf16, kind="ExternalInput").ap()
    wpt_d = nc.dram_tensor("wpt", [C, C], bf16, kind="ExternalInput").ap()
    # cvec columns: 0/1 = bq per o-tile, 2/3 = bk per o-tile, 4/5 = bp_eff per
    # c-tile.
    cvec_d = nc.dram_tensor("cvec", [128, 6], f32, kind="ExternalInput").ap()
    # cbf columns: 0:128 = 1/C bf16 (stats lhsT), 128 = 1.0 (rowsum lhsT),
    # 129:257 = 1.0 (row 0 used as K=1 lhsT for the 1/rowsum broadcast).
    cbf_d = nc.dram_tensor("cbf", [128, 257], bf16, kind="ExternalInput").ap()
    out_d = nc.dram_tensor("out", [C, N], f32, kind="ExternalOutput").ap()

    # m-tiles covered by each chunk: chunk j covers m in [off/128, (off+w)/128)
    def chunk_mtiles(ji):
        off, w = CHUNKS[ji]
        return range(off // 128, (off + w) // 128)

    with tile.TileContext(nc) as tc, ExitStack() as ctx:
        persist = ctx.enter_context(tc.tile_pool(name="persist", bufs=1))

        # DMA note: each descriptor covers one partition-row segment and the
        # queues are descriptor-rate-bound (~50ns each), so every load is
        # split into partition strips (full rows = max bytes per descriptor)
        # across queues rather than column chunks.
        def strip_load(dst, src, prows=32):
            P = dst.shape[0]
            for p in range(0, P, prows):
                pe = min(P, p + prows)
                nc.sync.dma_start(dst[p:pe, :], src[p:pe, :])

        # ---- constants: cbf gates the very first stats matmul ----------
        cbf = persist.tile([128, 257], bf16, tag="cbf", name="cbf")
        strip_load(cbf, cbf_d)

        # ---- x2 first: it gates k/v -> attention -----------------------
        x2sc = ctx.enter_context(tc.tile_pool(name="x2scope", bufs=1))
        x2_t = [
            x2sc.tile([128, N], bf16, tag=f"x2_{ct}", name=f"x2_{ct}")
            for ct in range(CT)
        ]
        for ct in range(CT):
            strip_load(x2_t[ct], x2_d[ct * 128 : (ct + 1) * 128, :])

        w_tiles = {}
        for nm, d in (("k", wkt_d), ("v", wvt_d)):
            for ct in range(CT):
                t = persist.tile([128, C], bf16, tag=f"w{nm}{ct}", name=f"w{nm}{ct}")
                strip_load(t, d[ct * 128 : (ct + 1) * 128, :])
                w_tiles[(nm, ct)] = t
        cvec = persist.tile([128, 6], f32, tag="cvec", name="cvec")
        strip_load(cvec, cvec_d)

        x1_t = [
            persist.tile([128, N], bf16, tag=f"x1_{ct}", name=f"x1_{ct}")
            for ct in range(CT)
        ]
        for ct in range(CT):
            strip_load(x1_t[ct], x1_d[ct * 128 : (ct + 1) * 128, :])

        for nm, d in (("q", wqt_d), ("p", wpt_d)):
            for ct in range(CT):
                t = persist.tile([128, C], bf16, tag=f"w{nm}{ct}", name=f"w{nm}{ct}")
                strip_load(t, d[ct * 128 : (ct + 1) * 128, :])
                w_tiles[(nm, ct)] = t

        # persistent intermediates
        k_t = [
            persist.tile([128, N], bf16, tag=f"k{ot}", name=f"k{ot}")
            for ot in range(CT)
        ]
        vT_t = [
            persist.tile([128, C], bf16, tag=f"vT{m}", name=f"vT{m}")
            for m in range(MT)
        ]
        xn1_t = [
            persist.tile([128, N], bf16, tag=f"xn1_{ct}", name=f"xn1_{ct}")
            for ct in range(CT)
        ]

        # ------------------------------------------------------------------
        # Pre-phase: per-chunk pipeline  stats -> xn -> k/vT   (x2 stream)
        # plus the x1 stats/xn stream (feeds q projections later).
        # ------------------------------------------------------------------
        with (
            tc.tile_pool(name="scr", bufs=3) as scr,
            tc.tile_pool(name="xnp", bufs=6) as xnp,
            tc.tile_pool(name="ps_st", bufs=2, space="PSUM") as ps_st,
            tc.tile_pool(name="ps_kv", bufs=2, space="PSUM") as ps_kv,
        ):

            def emit_stats_xn(tsel, ji, xsrc, xn_out):
                """stats + xn for (tensor tsel, chunk ji).

                xsrc: list of [128, N] bf16 tiles (per ct)
                xn_out: dict key (ct) -> (tile, col_off) destination slices
                """
                off, w = CHUNKS[ji]
                ub = ps_st.tile([128, 512], f32, tag="ub", name="ub")
                for ct in range(CT):
                    nc.tensor.matmul(
                        ub[:, :w],
                        cbf[:, 0:128],
                        xsrc[ct][:, off : off + w],
                        start=(ct == 0),
                        stop=(ct == CT - 1),
                    )
                ms = ps_st.tile([128, 512], f32, tag="ms", name="ms")
                for ct in range(CT):
                    xsq = scr.tile([128, 512], bf16, tag="xsq", name="xsq")
                    nc.gpsimd.tensor_mul(
                        xsq[:, :w],
                        xsrc[ct][:, off : off + w],
                        xsrc[ct][:, off : off + w],
                    )
                    nc.tensor.matmul(
                        ms[:, :w],
                        cbf[:, 0:128],
                        xsq[:, :w],
                        start=(ct == 0),
                        stop=(ct == CT - 1),
                    )
                usq = scr.tile([128, 512], f32, tag="usq", name="usq")
                nc.scalar.square(usq[:, :w], ub[:, :w])
                var = scr.tile([128, 512], f32, tag="var", name="var")
                nc.vector.scalar_tensor_tensor(
                    var[:, :w], ms[:, :w], EPS, usq[:, :w], ADD, SUB
                )
                std = scr.tile([128, 512], f32, tag="std", name="std")
                nc.scalar.activation(
                    std[:, :w], var[:, :w], mybir.ActivationFunctionType.Sqrt
                )
                rstd = scr.tile([128, 512], f32, tag=f"rstd{tsel}", name=f"rstd{tsel}")
                nc.vector.reciprocal_approx_fast(rstd[:, :w], std[:, :w])
                for ct in range(CT):
                    d = scr.tile([128, 512], f32, tag="xnd", name="xnd")
                    nc.vector.tensor_sub(
                        d[:, :w], xsrc[ct][:, off : off + w], ub[:, :w]
                    )
                    dst, dcol = xn_out[ct]
                    nc.vector.tensor_mul(
                        dst[:, dcol : dcol + w], d[:, :w], rstd[:, :w]
                    )

            xn2 = {}

            def emit_kv(ji):
                off, w = CHUNKS[ji]
                # k projection for this chunk of tokens
                for ot in range(CT):
                    ps = ps_kv.tile([128, 512], f32, tag="kv", name="kv")
                    for ct in range(CT):
                        nc.tensor.matmul(
                            ps[:, :w],
                            w_tiles[("k", ct)][:, ot * 128 : (ot + 1) * 128],
                            xn2[(ji, ct)][:, :w],
                            start=(ct == 0),
                            stop=(ct == CT - 1),
                        )
                    nc.vector.tensor_scalar_add(
                        k_t[ot][:, off : off + w], ps[:, :w], cvec[:, 2 + ot : 3 + ot]
                    )
                # vT for the m-tiles inside this chunk
                for m in chunk_mtiles(ji):
                    coff = m * 128 - off
                    ps = ps_kv.tile([128, C], f32, tag="kv", name="kv")
                    for ct in range(CT):
                        nc.tensor.matmul(
                            ps[:],
                            xn2[(ji, ct)][:, coff : coff + 128],
                            w_tiles[("v", ct)][:, :],
                            start=(ct == 0),
                            stop=(ct == CT - 1),
                        )
                    nc.scalar.copy(vT_t[m][:], ps[:])

            # x2 chunk 0 first (longest chain), then interleave x1 stats so
            # the PE always has stats matmuls to chew on while DVE chains run.
            for ji in range(NJ):
                for ct in range(CT):
                    t = xnp.tile([128, 512], bf16, tag="xn2", name=f"xn2_{ji}_{ct}")
                    xn2[(ji, ct)] = t
                emit_stats_xn(1, ji, x2_t, {ct: (xn2[(ji, ct)], 0) for ct in range(CT)})
                emit_kv(ji)
                # x1 stream trails: its xn goes to the persistent xn1 tiles
                emit_stats_xn(
                    0, ji, x1_t, {ct: (xn1_t[ct], CHUNKS[ji][0]) for ct in range(CT)}
                )

        # ------------------------------------------------------------------
        # Attention: per q-chunk; q projected one chunk ahead; epilogue
        # (normalize, Wp projection, residual, DMA out) inside the loop.
        # ------------------------------------------------------------------
        with (
            tc.tile_pool(name="qch", bufs=4) as qch,
            tc.tile_pool(name="pt", bufs=4) as pt_pool,
            tc.tile_pool(name="oup", bufs=4) as oup,
            tc.tile_pool(name="invp", bufs=2) as invp,
            tc.tile_pool(name="outp", bufs=4) as outp,
            tc.tile_pool(name="ps_pj", bufs=2, space="PSUM") as ps_pj,
            tc.tile_pool(name="ps_qk", bufs=2, space="PSUM") as ps_qk,
            tc.tile_pool(name="ps_o", bufs=2, space="PSUM") as ps_o,
            tc.tile_pool(name="ps_rs", bufs=1, space="PSUM") as ps_rs,
            tc.tile_pool(name="ps_rsb", bufs=1, space="PSUM") as ps_rsb,
        ):
            q_ch = {}

            def emit_qproj(ji):
                off, w = CHUNKS[ji]
                for ot in range(CT):
                    ps = ps_pj.tile([128, 512], f32, tag="pj", name="pj")
                    for ct in range(CT):
                        nc.tensor.matmul(
                            ps[:, :w],
                            w_tiles[("q", ct)][:, ot * 128 : (ot + 1) * 128],
                            xn1_t[ct][:, off : off + w],
                            start=(ct == 0),
                            stop=(ct == CT - 1),
                        )
                    qt = qch.tile([128, 512], bf16, tag="q", name=f"q{ji}_{ot}")
                    nc.vector.tensor_scalar_add(
                        qt[:, :w], ps[:, :w], cvec[:, 0 + ot : 1 + ot]
                    )
                    q_ch[(ji, ot)] = qt

            emit_qproj(0)
            for ji, (off, w) in enumerate(CHUNKS):
                if ji + 1 < NJ:
                    emit_qproj(ji + 1)
                st = {}

                def emit_qk(m):
                    ps = ps_qk.tile([128, 512], f32, tag="st", name="st")
                    for ot in range(CT):
                        nc.tensor.matmul(
                            ps[:, :w],
                            k_t[ot][:, m * 128 : (m + 1) * 128],
                            q_ch[(ji, ot)][:, :w],
                            start=(ot == 0),
                            stop=(ot == CT - 1),
                        )
                    st[m] = ps

                o_ps = [
                    ps_o.tile([128, 512], f32, tag="o", name="o") for _ in range(CT)
                ]
                rs_ps = ps_rs.tile([1, 512], f32, tag="rsp", name="rsp")

                emit_qk(0)
                for m in range(MT):
                    if m + 1 < MT:
                        emit_qk(m + 1)
                    pt = pt_pool.tile([128, 512], bf16, tag="pt", name=f"pt{m}")
                    nc.scalar.activation(
                        pt[:, :w], st[m][:, :w], mybir.ActivationFunctionType.Exp
                    )
                    del st[m]
                    for c in range(CT):
                        nc.tensor.matmul(
                            o_ps[c][:, :w],
                            vT_t[m][:, c * 128 : (c + 1) * 128],
                            pt[:, :w],
                            start=(m == 0),
                            stop=(m == MT - 1),
                        )
                    nc.tensor.matmul(
                        rs_ps[:, :w],
                        cbf[:, 128:129],
                        pt[:, :w],
                        start=(m == 0),
                        stop=(m == MT - 1),
                    )

                # ---- chunk epilogue -----------------------------------
                # rowsum row -> bf16 -> K=1 ones-matmul broadcast to 128
                # partitions -> reciprocal -> fused into the ou eviction.
                rs_row = invp.tile([1, 512], bf16, tag="rsrow", name="rsrow")
                nc.vector.tensor_copy(rs_row[0:1, :w], rs_ps[0:1, :w])
                rs_b = ps_rsb.tile([128, 512], f32, tag="rsb", name="rsb")
                nc.tensor.matmul(
                    rs_b[:, :w], cbf[0:1, 129:257], rs_row[0:1, :w],
                    start=True, stop=True,
                )
                inv_b = invp.tile([128, 512], f32, tag="invb", name="invb")
                nc.vector.reciprocal_approx_fast(inv_b[:, :w], rs_b[:, :w])

                ou = []
                for c in range(CT):
                    t = oup.tile([128, 512], bf16, tag="ou", name=f"ou{c}")
                    nc.vector.tensor_mul(t[:, :w], o_ps[c][:, :w], inv_b[:, :w])
                    ou.append(t)

                for ct in range(CT):
                    ps = ps_pj.tile([128, 512], f32, tag="pj", name="pj")
                    for ci in range(CT):
                        nc.tensor.matmul(
                            ps[:, :w],
                            w_tiles[("p", ci)][:, ct * 128 : (ct + 1) * 128],
                            ou[ci][:, :w],
                            start=(ci == 0),
                            stop=(ci == CT - 1),
                        )
                    ot_t = outp.tile([128, 512], f32, tag="outt", name=f"out{ct}")
                    nc.vector.scalar_tensor_tensor(
                        ot_t[:, :w],
                        ps[:, :w],
                        cvec[:, 4 + ct : 5 + ct],
                        x1_t[ct][:, off : off + w],
                        ADD,
                        ADD,
                    )
                    for p in range(0, 128, 32):
                        nc.sync.dma_start(
                            out_d[ct * 128 + p : ct * 128 + p + 32, off : off + w],
                            ot_t[p : p + 32, :w],
                        )

    nc.compile()
    return nc


def _host_prep(inputs):
    f = lambda k: np.asarray(inputs[k], dtype=np.float32)
    Wq, Wk, Wv, Wp = f("Wq"), f("Wk"), f("Wv"), f("Wp")
    bq, bk, bv, bp = f("bq"), f("bk"), f("bv"), f("bp")
    w_nq, b_nq, w_nkv, b_nkv = f("w_nq"), f("b_nq"), f("w_nkv"), f("b_nkv")

    Wq_eff = Wq * w_nq[None, :] * SCALE
    bq_eff = SCALE * (bq + Wq @ b_nq)
    Wk_eff = Wk * w_nkv[None, :]
    bk_eff = bk + Wk @ b_nkv
    Wv_eff = Wv * w_nkv[None, :]
    bv_eff = bv + Wv @ b_nkv
    bp_eff = bp + Wp @ bv_eff  # v bias folded through softmax + Wp

    wqt = np.ascontiguousarray(Wq_eff.T).astype(BF16)
    wkt = np.ascontiguousarray(Wk_eff.T).astype(BF16)
    wvt = np.ascontiguousarray(Wv_eff.T).astype(BF16)
    wpt = np.ascontiguousarray(Wp.T).astype(BF16)

    cvec = np.zeros((128, 6), np.float32)
    cvec[:, 0] = bq_eff[0:128]
    cvec[:, 1] = bq_eff[128:256]
    cvec[:, 2] = bk_eff[0:128]
    cvec[:, 3] = bk_eff[128:256]
    cvec[:, 4] = bp_eff[0:128]
    cvec[:, 5] = bp_eff[128:256]

    cbf = np.zeros((128, 257), np.float32)
    cbf[:, 0:128] = 1.0 / C
    cbf[:, 128] = 1.0
    cbf[:, 129:257] = 1.0
    cbf = cbf.astype(BF16)

    return dict(wqt=wqt, wkt=wkt, wvt=wvt, wpt=wpt, cvec=cvec, cbf=cbf)


def _maybe_patch_ldw_opt():
    if os.environ.get("BASS_LDW_OPT", "0") != "1":
        return
    import concourse.bass_utils as bu
    if getattr(bu, "_ldw_patch", False):
        return
    orig = bu.run_command
    def patched(argv, **kw):
        if isinstance(argv, list):
            argv = [a.replace("--enable-ldw-opt=false", "--enable-ldw-opt=true") for a in argv]
        return orig(argv, **kw)
    bu.run_command = patched
    bu._ldw_patch = True


def kernel(**inputs):
    global last_results
    _maybe_patch_ldw_opt()
    from concourse.bass_utils import run_bass_kernel_spmd

    if "nc" not in _cache:
        _cache["nc"] = _build_program()
    nc = _cache["nc"]

    shared = _host_prep(inputs)
    x1 = np.asarray(inputs["x1"], dtype=np.float32).reshape(B, C, N).astype(BF16)
    x2 = np.asarray(inputs["x2"], dtype=np.float32).reshape(B, C, N).astype(BF16)

    in_maps = []
    for b in range(B):
        m = dict(shared)
        m["x1"] = np.ascontiguousarray(x1[b])
        m["x2"] = np.ascontiguousarray(x2[b])
        in_maps.append(m)

    trace = os.environ.get("BASS_KERNEL_TRACE", "0") == "1"
    res = run_bass_kernel_spmd(
        nc, in_maps, core_ids=list(range(B)), trace=trace
    )
    last_results = res
    out = np.stack([res.results[b]["out"].reshape(C, H, W) for b in range(B)])
    return out.astype(np.float32)


# revision 26
# speedup vs baseline: 1.1477x; 1.0292x over previous
"""Trainium2 Bass kernel for nn_CrossAttentionBlock (B=8, C=256, H=W=48).

Sharding: data-parallel over batch B — one batch per NeuronCore (8 cores).

Per-core math (x: [C=256, N=2304] f32):
  LayerNorm over C folded into projection weights on host:
      W_eff = W * w_n[None,:],  b_eff = b + W @ b_n
  attention SCALE folded into Wq_eff / bq_eff.
  v bias folded into the output bias (softmax rows sum to one, so
  attn@(v+bv) = attn@v + bv, hence bp_eff = bp + Wp@bv_eff).

  Activations are host-cast to bf16 and DMA'd as [32, N] partition strips
  (the DMA queues are descriptor-rate-bound; full-row descriptors maximize
  bytes per descriptor).  Stats: mean via a 1/C-ones bf16 matmul on x;
  mean-square via a Pool-computed x*x fed to a second ones-matmul.
  xn = (x-u)*rstd on the DVE, emitted as bf16.

  Attention is computed transposed:  St[m,n] = sum_o k[o,m] q[o,n]
  so softmax normalization runs over the *partition* axis m:
    - no row-max subtraction (logits bounded ~21, exp safe in f32)
    - P = exp(St) (ScalarE, PSUM->SBUF bf16 eviction)
    - rowsum[n] = sum_m P[m,n] via an M=1 ones-matmul accumulated across m
    - 1/rowsum applied AFTER the output projection (scaling commutes
      with Wp); the reciprocal row is partition-broadcast by the Pool
      engine and fused into the PSUM->SBUF eviction of the attention
      output, and the Wp projection + residual + DMA-out run per chunk
      inside the attention loop so there is no serial tail.
  v is produced directly transposed (vT[m,o] = sum_c xn2[c,m] WvT[c,o])
  so P.V contracts over m on partitions with zero PE transposes.
"""

import os
import sys
import types
import ctypes
import contextlib

sys.path.insert(0, "/opt/trn_rl_repo")

import numpy as np
import ml_dtypes

# ---------------------------------------------------------------------------
# NTFF profile hook stub (antenv.axon_hooks is absent in this container; the
# ctypes shim mirrors trn_agent_boot). Only used when tracing is requested.
# ---------------------------------------------------------------------------


def _ntff_profile_via_ctypes(so_path):
    try:
        lib = ctypes.CDLL(so_path)
    except OSError:
        return None
    if not hasattr(lib, "axon_start_nrt_profile"):
        return None
    lib.axon_start_nrt_profile.argtypes = [
        ctypes.POINTER(ctypes.c_int64),
        ctypes.c_size_t,
    ]
    lib.axon_start_nrt_profile.restype = ctypes.c_int64
    lib.axon_stop_nrt_profile.argtypes = [ctypes.c_char_p]
    lib.axon_stop_nrt_profile.restype = ctypes.c_int64

    @contextlib.contextmanager
    def _hook(output_dir, device_ids):
        import jax

        jax.devices()
        if device_ids:
            ids = (ctypes.c_int64 * len(device_ids))(*device_ids)
            rc = lib.axon_start_nrt_profile(ids, len(device_ids))
        else:
            rc = lib.axon_start_nrt_profile(None, 0)
        if rc != 0:
            raise RuntimeError(f"axon_start_nrt_profile rc={rc}")
        try:
            yield
        finally:
            n = lib.axon_stop_nrt_profile(str(output_dir).encode())
            print(f"profile: {n} file(s) written to {output_dir}", file=sys.stderr)

    return _hook


if "antenv.axon_hooks" not in sys.modules:
    _hook = _ntff_profile_via_ctypes("/opt/axon/libaxon_pjrt.so")
    _mod = types.ModuleType("antenv.axon_hooks")
    _mod.get_axon_ntff_profile_hook = lambda: _hook
    sys.modules["antenv.axon_hooks"] = _mod

# ---------------------------------------------------------------------------

B, C, H, W = 8, 256, 48, 48
N = H * W  # 2304
SCALE = (C // 8) ** (-0.5)
EPS = 1e-6
CT = C // 128  # 2 channel tiles
MT = N // 128  # 18 m (key-token) tiles
CHUNKS = [(0, 512), (512, 512), (1024, 512), (1536, 512), (2048, 256)]
NJ = len(CHUNKS)

BF16 = ml_dtypes.bfloat16

_cache = {}
last_results = None  # BassKernelResults of the most recent run (for test.py)


def _build_program():
    import concourse.bacc as bacc
    import concourse.tile as tile
    import concourse.mybir as mybir
    from contextlib import ExitStack

    f32 = mybir.dt.float32
    bf16 = mybir.dt.bfloat16
    ADD = mybir.AluOpType.add
    SUB = mybir.AluOpType.subtract

    nc = bacc.Bacc("TRN2", target_bir_lowering=False, debug=False)

    x1_d = nc.dram_tensor("x1", [C, N], bf16, kind="ExternalInput").ap()
    x2_d = nc.dram_tensor("x2", [C, N], bf16, kind="ExternalInput").ap()
    wqt_d = nc.dram_tensor("wqt", [C, C], bf16, kind="ExternalInput").ap()
    wkt_d = nc.dram_tensor("wkt", [C, C], bf16, kind="ExternalInput").ap()
    wvt_d = nc.dram_tensor("wvt", [C, C], bf16, kind="ExternalInput").ap()
    wpt_d = nc.dram_tensor("wpt", [C, C], bf16, kind="ExternalInput").ap()
    # cvec columns: 0/1 = bq per o-tile, 2/3 = bk per o-tile, 4/5 = bp_eff per
    # c-tile.  (The 1/C and ones constant blocks are memset on-device.)
    cvec_d = nc.dram_tensor("cvec", [128, 6], f32, kind="ExternalInput").ap()
    out_d = nc.dram_tensor("out", [C, N], f32, kind="ExternalOutput").ap()

    # m-tiles covered by each chunk: chunk j covers m in [off/128, (off+w)/128)
    def chunk_mtiles(ji):
        off, w = CHUNKS[ji]
        return range(off // 128, (off + w) // 128)

    with tile.TileContext(nc) as tc, ExitStack() as ctx:
        persist = ctx.enter_context(tc.tile_pool(name="persist", bufs=1))

        # DMA notes: (1) each descriptor covers one partition-row segment and
        # the queues are descriptor-rate-bound (~50ns each), so loads are
        # split into partition strips (full rows = max bytes per descriptor);
        # (2) each dma_start costs ~600ns of *issue* time on its engine's
        # sequencer, so issues are spread across the four idle sequencers.
        def strip_load(eng, dst, src, prows):
            P = dst.shape[0]
            for p in range(0, P, prows):
                pe = min(P, p + prows)
                eng.dma_start(dst[p:pe, :], src[p:pe, :])

        # ---- constants built on-device (no DMA) ------------------------
        invC = persist.tile([128, 128], bf16, tag="invC", name="invC")
        nc.vector.memset(invC[:], 1.0 / C)
        onesb = persist.tile([128, 128], bf16, tag="onesb", name="onesb")
        nc.gpsimd.memset(onesb[:], 1.0)

        # ---- x2 first: it gates k/v -> attention -----------------------
        x2sc = ctx.enter_context(tc.tile_pool(name="x2scope", bufs=1))
        x2_t = [
            x2sc.tile([128, N], bf16, tag=f"x2_{ct}", name=f"x2_{ct}")
            for ct in range(CT)
        ]
        strip_load(nc.sync, x2_t[0], x2_d[0:128, :], 32)
        strip_load(nc.scalar, x2_t[1], x2_d[128:256, :], 32)

        cvec = persist.tile([128, 6], f32, tag="cvec", name="cvec")
        nc.sync.dma_start(cvec[:], cvec_d[:, :])

        w_tiles = {}
        for (nm, d), eng in ((("k", wkt_d), nc.scalar), (("v", wvt_d), nc.gpsimd)):
            for ct in range(CT):
                t = persist.tile([128, C], bf16, tag=f"w{nm}{ct}", name=f"w{nm}{ct}")
                strip_load(eng, t, d[ct * 128 : (ct + 1) * 128, :], 64)
                w_tiles[(nm, ct)] = t

        x1_t = [
            persist.tile([128, N], bf16, tag=f"x1_{ct}", name=f"x1_{ct}")
            for ct in range(CT)
        ]
        strip_load(nc.sync, x1_t[0], x1_d[0:128, :], 64)
        strip_load(nc.gpsimd, x1_t[1], x1_d[128:256, :], 64)

        for nm, d in (("q", wqt_d), ("p", wpt_d)):
            for ct in range(CT):
                t = persist.tile([128, C], bf16, tag=f"w{nm}{ct}", name=f"w{nm}{ct}")
                nc.sync.dma_start(t[:], d[ct * 128 : (ct + 1) * 128, :])
                w_tiles[(nm, ct)] = t

        # persistent intermediates
        k_t = [
            persist.tile([128, N], bf16, tag=f"k{ot}", name=f"k{ot}")
            for ot in range(CT)
        ]
        vT_t = [
            persist.tile([128, C], bf16, tag=f"vT{m}", name=f"vT{m}")
            for m in range(MT)
        ]
        xn1_t = [
            persist.tile([128, N], bf16, tag=f"xn1_{ct}", name=f"xn1_{ct}")
            for ct in range(CT)
        ]

        # ------------------------------------------------------------------
        # Pre-phase: per-chunk pipeline  stats -> xn -> k/vT   (x2 stream)
        # plus the x1 stats/xn stream (feeds q projections later).
        # ------------------------------------------------------------------
        with (
            tc.tile_pool(name="scr", bufs=3) as scr,
            tc.tile_pool(name="xnp", bufs=6) as xnp,
            tc.tile_pool(name="ps_st", bufs=2, space="PSUM") as ps_st,
            tc.tile_pool(name="ps_kv", bufs=2, space="PSUM") as ps_kv,
        ):

            def emit_stats_xn(tsel, ji, xsrc, xn_out):
                """stats + xn for (tensor tsel, chunk ji).

                xsrc: list of [128, N] bf16 tiles (per ct)
                xn_out: dict key (ct) -> (tile, col_off) destination slices
                """
                off, w = CHUNKS[ji]
                ub = ps_st.tile([128, 512], f32, tag="ub", name="ub")
                for ct in range(CT):
                    nc.tensor.matmul(
                        ub[:, :w],
                        invC[:],
                        xsrc[ct][:, off : off + w],
                        start=(ct == 0),
                        stop=(ct == CT - 1),
                    )
                ms = ps_st.tile([128, 512], f32, tag="ms", name="ms")
                for ct in range(CT):
                    xsq = scr.tile([128, 512], bf16, tag="xsq", name="xsq")
                    nc.gpsimd.tensor_mul(
                        xsq[:, :w],
                        xsrc[ct][:, off : off + w],
                        xsrc[ct][:, off : off + w],
                    )
                    nc.tensor.matmul(
                        ms[:, :w],
                        invC[:],
                        xsq[:, :w],
                        start=(ct == 0),
                        stop=(ct == CT - 1),
                    )
                usq = scr.tile([128, 512], f32, tag="usq", name="usq")
                nc.scalar.square(usq[:, :w], ub[:, :w])
                var = scr.tile([128, 512], f32, tag="var", name="var")
                nc.vector.scalar_tensor_tensor(
                    var[:, :w], ms[:, :w], EPS, usq[:, :w], ADD, SUB
                )
                std = scr.tile([128, 512], f32, tag="std", name="std")
                nc.scalar.activation(
                    std[:, :w], var[:, :w], mybir.ActivationFunctionType.Sqrt
                )
                rstd = scr.tile([128, 512], f32, tag=f"rstd{tsel}", name=f"rstd{tsel}")
                nc.vector.reciprocal_approx_fast(rstd[:, :w], std[:, :w])
                for ct in range(CT):
                    d = scr.tile([128, 512], f32, tag="xnd", name="xnd")
                    nc.vector.tensor_sub(
                        d[:, :w], xsrc[ct][:, off : off + w], ub[:, :w]
                    )
                    dst, dcol = xn_out[ct]
                    nc.vector.tensor_mul(
                        dst[:, dcol : dcol + w], d[:, :w], rstd[:, :w]
                    )

            xn2 = {}

            def emit_kv(ji):
                off, w = CHUNKS[ji]
                # k projection for this chunk of tokens
                for ot in range(CT):
                    ps = ps_kv.tile([128, 512], f32, tag="kv", name="kv")
                    for ct in range(CT):
                        nc.tensor.matmul(
                            ps[:, :w],
                            w_tiles[("k", ct)][:, ot * 128 : (ot + 1) * 128],
                            xn2[(ji, ct)][:, :w],
                            start=(ct == 0),
                            stop=(ct == CT - 1),
                        )
                    nc.vector.tensor_scalar_add(
                        k_t[ot][:, off : off + w], ps[:, :w], cvec[:, 2 + ot : 3 + ot]
                    )
                # vT for the m-tiles inside this chunk
                for m in chunk_mtiles(ji):
                    coff = m * 128 - off
                    ps = ps_kv.tile([128, C], f32, tag="kv", name="kv")
                    for ct in range(CT):
                        nc.tensor.matmul(
                            ps[:],
                            xn2[(ji, ct)][:, coff : coff + 128],
                            w_tiles[("v", ct)][:, :],
                            start=(ct == 0),
                            stop=(ct == CT - 1),
                        )
                    nc.scalar.copy(vT_t[m][:], ps[:])

            # x2 chunk 0 first (longest chain), then interleave x1 stats so
            # the PE always has stats matmuls to chew on while DVE chains run.
            for ji in range(NJ):
                for ct in range(CT):
                    t = xnp.tile([128, 512], bf16, tag="xn2", name=f"xn2_{ji}_{ct}")
                    xn2[(ji, ct)] = t
                emit_stats_xn(1, ji, x2_t, {ct: (xn2[(ji, ct)], 0) for ct in range(CT)})
                emit_kv(ji)
                # x1 stream trails: its xn goes to the persistent xn1 tiles
                emit_stats_xn(
                    0, ji, x1_t, {ct: (xn1_t[ct], CHUNKS[ji][0]) for ct in range(CT)}
                )

        # ------------------------------------------------------------------
        # Attention: per q-chunk; q projected one chunk ahead; epilogue
        # (normalize, Wp projection, residual, DMA out) inside the loop.
        # ------------------------------------------------------------------
        with (
            tc.tile_pool(name="qch", bufs=4) as qch,
            tc.tile_pool(name="pt", bufs=4) as pt_pool,
            tc.tile_pool(name="oup", bufs=4) as oup,
            tc.tile_pool(name="invp", bufs=2) as invp,
            tc.tile_pool(name="outp", bufs=4) as outp,
            tc.tile_pool(name="ps_pj", bufs=2, space="PSUM") as ps_pj,
            tc.tile_pool(name="ps_qk", bufs=2, space="PSUM") as ps_qk,
            tc.tile_pool(name="ps_o", bufs=3, space="PSUM") as ps_o,
            tc.tile_pool(name="ps_rs", bufs=1, space="PSUM") as ps_rs,
        ):
            q_ch = {}

            def emit_qproj(ji):
                off, w = CHUNKS[ji]
                for ot in range(CT):
                    ps = ps_pj.tile([128, 512], f32, tag="pj", name="pj")
                    for ct in range(CT):
                        nc.tensor.matmul(
                            ps[:, :w],
                            w_tiles[("q", ct)][:, ot * 128 : (ot + 1) * 128],
                            xn1_t[ct][:, off : off + w],
                            start=(ct == 0),
                            stop=(ct == CT - 1),
                        )
                    qt = qch.tile([128, 512], bf16, tag="q", name=f"q{ji}_{ot}")
                    nc.vector.tensor_scalar_add(
                        qt[:, :w], ps[:, :w], cvec[:, 0 + ot : 1 + ot]
                    )
                    q_ch[(ji, ot)] = qt

            emit_qproj(0)
            for ji, (off, w) in enumerate(CHUNKS):
                if ji + 1 < NJ:
                    emit_qproj(ji + 1)
                st = {}

                def emit_qk(m):
                    ps = ps_qk.tile([128, 512], f32, tag="st", name="st")
                    for ot in range(CT):
                        nc.tensor.matmul(
                            ps[:, :w],
                            k_t[ot][:, m * 128 : (m + 1) * 128],
                            q_ch[(ji, ot)][:, :w],
                            start=(ot == 0),
                            stop=(ot == CT - 1),
                        )
                    st[m] = ps

                o_ps = [
                    ps_o.tile([128, 512], f32, tag="o", name="o") for _ in range(CT)
                ]
                rs_ps = ps_rs.tile([1, 512], f32, tag="rsp", name="rsp")

                emit_qk(0)
                for m in range(MT):
                    if m + 1 < MT:
                        emit_qk(m + 1)
                    pt = pt_pool.tile([128, 512], bf16, tag="pt", name=f"pt{m}")
                    nc.scalar.activation(
                        pt[:, :w], st[m][:, :w], mybir.ActivationFunctionType.Exp
                    )
                    del st[m]
                    for c in range(CT):
                        nc.tensor.matmul(
                            o_ps[c][:, :w],
                            vT_t[m][:, c * 128 : (c + 1) * 128],
                            pt[:, :w],
                            start=(m == 0),
                            stop=(m == MT - 1),
                        )
                    nc.tensor.matmul(
                        rs_ps[:, :w],
                        onesb[:, 0:1],
                        pt[:, :w],
                        start=(m == 0),
                        stop=(m == MT - 1),
                    )

                # ---- chunk epilogue -----------------------------------
                # rowsum row -> bf16 -> K=1 ones-matmul broadcast to 128
                # partitions -> reciprocal -> fused into the ou eviction.
                rs_row = invp.tile([1, 512], bf16, tag="rsrow", name="rsrow")
                nc.vector.tensor_copy(rs_row[0:1, :w], rs_ps[0:1, :w])
                rs_b = ps_qk.tile([128, 512], f32, tag="st", name="rsb")
                nc.tensor.matmul(
                    rs_b[:, :w], onesb[0:1, 0:128], rs_row[0:1, :w],
                    start=True, stop=True,
                )
                inv_b = invp.tile([128, 512], f32, tag="invb", name="invb")
                nc.vector.reciprocal_approx_fast(inv_b[:, :w], rs_b[:, :w])

                ou = []
                for c in range(CT):
                    t = oup.tile([128, 512], bf16, tag="ou", name=f"ou{c}")
                    nc.vector.tensor_mul(t[:, :w], o_ps[c][:, :w], inv_b[:, :w])
                    ou.append(t)

                for ct in range(CT):
                    ps = ps_pj.tile([128, 512], f32, tag="pj", name="pj")
                    for ci in range(CT):
                        nc.tensor.matmul(
                            ps[:, :w],
                            w_tiles[("p", ci)][:, ct * 128 : (ct + 1) * 128],
                            ou[ci][:, :w],
                            start=(ci == 0),
                            stop=(ci == CT - 1),
                        )
                    ot_t = outp.tile([128, 512], f32, tag="outt", name=f"out{ct}")
                    nc.vector.scalar_tensor_tensor(
                        ot_t[:, :w],
                        ps[:, :w],
                        cvec[:, 4 + ct : 5 + ct],
                        x1_t[ct][:, off : off + w],
                        ADD,
                        ADD,
                    )
                    if ji + 1 < NJ:
                        nc.sync.dma_start(
                            out_d[ct * 128 : (ct + 1) * 128, off : off + w],
                            ot_t[:, :w],
                        )
                    else:
                        # last chunk is latency-critical: strip across queues
                        for p in range(0, 128, 64):
                            nc.sync.dma_start(
                                out_d[ct * 128 + p : ct * 128 + p + 64, off : off + w],
                                ot_t[p : p + 64, :w],
                            )

    nc.compile()
    return nc


def _host_prep(inputs):
    f = lambda k: np.asarray(inputs[k], dtype=np.float32)
    Wq, Wk, Wv, Wp = f("Wq"), f("Wk"), f("Wv"), f("Wp")
    bq, bk, bv, bp = f("bq"), f("bk"), f("bv"), f("bp")
    w_nq, b_nq, w_nkv, b_nkv = f("w_nq"), f("b_nq"), f("w_nkv"), f("b_nkv")

    Wq_eff = Wq * w_nq[None, :] * SCALE
    bq_eff = SCALE * (bq + Wq @ b_nq)
    Wk_eff = Wk * w_nkv[None, :]
    bk_eff = bk + Wk @ b_nkv
    Wv_eff = Wv * w_nkv[None, :]
    bv_eff = bv + Wv @ b_nkv
    bp_eff = bp + Wp @ bv_eff  # v bias folded through softmax + Wp

    wqt = np.ascontiguousarray(Wq_eff.T).astype(BF16)
    wkt = np.ascontiguousarray(Wk_eff.T).astype(BF16)
    wvt = np.ascontiguousarray(Wv_eff.T).astype(BF16)
    wpt = np.ascontiguousarray(Wp.T).astype(BF16)

    cvec = np.zeros((128, 6), np.float32)
    cvec[:, 0] = bq_eff[0:128]
    cvec[:, 1] = bq_eff[128:256]
    cvec[:, 2] = bk_eff[0:128]
    cvec[:, 3] = bk_eff[128:256]
    cvec[:, 4] = bp_eff[0:128]
    cvec[:, 5] = bp_eff[128:256]

    return dict(wqt=wqt, wkt=wkt, wvt=wvt, wpt=wpt, cvec=cvec)


def _maybe_patch_ldw_opt():
    if os.environ.get("BASS_LDW_OPT", "0") != "1":
        return
    import concourse.bass_utils as bu
    if getattr(bu, "_ldw_patch", False):
        return
    orig = bu.run_command
    def patched(argv, **kw):
        if isinstance(argv, list):
            argv = [a.replace("--enable-ldw-opt=false", "--enable-ldw-opt=true") for a in argv]
        return orig(argv, **kw)
    bu.run_command = patched
    bu._ldw_patch = True


def kernel(**inputs):
    global last_results
    _maybe_patch_ldw_opt()
    from concourse.bass_utils import run_bass_kernel_spmd

    if "nc" not in _cache:
        _cache["nc"] = _build_program()
    nc = _cache["nc"]

    shared = _host_prep(inputs)
    x1 = np.asarray(inputs["x1"], dtype=np.float32).reshape(B, C, N).astype(BF16)
    x2 = np.asarray(inputs["x2"], dtype=np.float32).reshape(B, C, N).astype(BF16)

    in_maps = []
    for b in range(B):
        m = dict(shared)
        m["x1"] = np.ascontiguousarray(x1[b])
        m["x2"] = np.ascontiguousarray(x2[b])
        in_maps.append(m)

    trace = os.environ.get("BASS_KERNEL_TRACE", "0") == "1"
    res = run_bass_kernel_spmd(
        nc, in_maps, core_ids=list(range(B)), trace=trace
    )
    last_results = res
    out = np.stack([res.results[b]["out"].reshape(C, H, W) for b in range(B)])
    return out.astype(np.float32)


# revision 29
# speedup vs baseline: 1.2560x; 1.0943x over previous
"""Trainium2 Bass kernel for nn_CrossAttentionBlock (B=8, C=256, H=W=48).

Sharding: data-parallel over batch B — one batch per NeuronCore (8 cores).

Per-core math (x: [C=256, N=2304] f32):
  LayerNorm over C folded into projection weights on host:
      W_eff = W * w_n[None,:],  b_eff = b + W @ b_n
  attention SCALE folded into Wq_eff / bq_eff.
  v bias folded into the output bias (softmax rows sum to one, so
  attn@(v+bv) = attn@v + bv, hence bp_eff = bp + Wp@bv_eff).

  Activations are host-cast to bf16 and DMA'd as [32, N] partition strips
  (the DMA queues are descriptor-rate-bound; full-row descriptors maximize
  bytes per descriptor).  Stats: mean via a 1/C-ones bf16 matmul on x;
  mean-square via a Pool-computed x*x fed to a second ones-matmul.
  xn = (x-u)*rstd on the DVE, emitted as bf16.

  Attention is computed transposed:  St[m,n] = sum_o k[o,m] q[o,n]
  so softmax normalization runs over the *partition* axis m:
    - no row-max subtraction (logits bounded ~21, exp safe in f32)
    - P = exp(St) (ScalarE, PSUM->SBUF bf16 eviction)
    - rowsum[n] = sum_m P[m,n] via an M=1 ones-matmul accumulated across m
    - 1/rowsum applied AFTER the output projection (scaling commutes
      with Wp); the reciprocal row is partition-broadcast by the Pool
      engine and fused into the PSUM->SBUF eviction of the attention
      output, and the Wp projection + residual + DMA-out run per chunk
      inside the attention loop so there is no serial tail.
  v is produced directly transposed (vT[m,o] = sum_c xn2[c,m] WvT[c,o])
  so P.V contracts over m on partitions with zero PE transposes.
"""

import os
import sys
import types
import ctypes
import contextlib

sys.path.insert(0, "/opt/trn_rl_repo")

import numpy as np
import ml_dtypes

# ---------------------------------------------------------------------------
# NTFF profile hook stub (antenv.axon_hooks is absent in this container; the
# ctypes shim mirrors trn_agent_boot). Only used when tracing is requested.
# ---------------------------------------------------------------------------


def _ntff_profile_via_ctypes(so_path):
    try:
        lib = ctypes.CDLL(so_path)
    except OSError:
        return None
    if not hasattr(lib, "axon_start_nrt_profile"):
        return None
    lib.axon_start_nrt_profile.argtypes = [
        ctypes.POINTER(ctypes.c_int64),
        ctypes.c_size_t,
    ]
    lib.axon_start_nrt_profile.restype = ctypes.c_int64
    lib.axon_stop_nrt_profile.argtypes = [ctypes.c_char_p]
    lib.axon_stop_nrt_profile.restype = ctypes.c_int64

    @contextlib.contextmanager
    def _hook(output_dir, device_ids):
        import jax

        jax.devices()
        if device_ids:
            ids = (ctypes.c_int64 * len(device_ids))(*device_ids)
            rc = lib.axon_start_nrt_profile(ids, len(device_ids))
        else:
            rc = lib.axon_start_nrt_profile(None, 0)
        if rc != 0:
            raise RuntimeError(f"axon_start_nrt_profile rc={rc}")
        try:
            yield
        finally:
            n = lib.axon_stop_nrt_profile(str(output_dir).encode())
            print(f"profile: {n} file(s) written to {output_dir}", file=sys.stderr)

    return _hook


if "antenv.axon_hooks" not in sys.modules:
    _hook = _ntff_profile_via_ctypes("/opt/axon/libaxon_pjrt.so")
    _mod = types.ModuleType("antenv.axon_hooks")
    _mod.get_axon_ntff_profile_hook = lambda: _hook
    sys.modules["antenv.axon_hooks"] = _mod

# ---------------------------------------------------------------------------

B, C, H, W = 8, 256, 48, 48
N = H * W  # 2304
SCALE = (C // 8) ** (-0.5)
EPS = 1e-6
CT = C // 128  # 2 channel tiles
MT = N // 128  # 18 m (key-token) tiles
CHUNKS = [(0, 512), (512, 512), (1024, 512), (1536, 512), (2048, 256)]
NJ = len(CHUNKS)

BF16 = ml_dtypes.bfloat16

_cache = {}
last_results = None  # BassKernelResults of the most recent run (for test.py)


def _build_program():
    import concourse.bacc as bacc
    import concourse.tile as tile
    import concourse.mybir as mybir
    from contextlib import ExitStack

    f32 = mybir.dt.float32
    bf16 = mybir.dt.bfloat16
    ADD = mybir.AluOpType.add
    SUB = mybir.AluOpType.subtract

    nc = bacc.Bacc("TRN2", target_bir_lowering=False, debug=False)

    x1_d = nc.dram_tensor("x1", [C, N], bf16, kind="ExternalInput").ap()
    x2_d = nc.dram_tensor("x2", [C, N], bf16, kind="ExternalInput").ap()
    wqt_d = nc.dram_tensor("wqt", [C, C], bf16, kind="ExternalInput").ap()
    wkt_d = nc.dram_tensor("wkt", [C, C], bf16, kind="ExternalInput").ap()
    wvt_d = nc.dram_tensor("wvt", [C, C], bf16, kind="ExternalInput").ap()
    wpt_d = nc.dram_tensor("wpt", [C, C], bf16, kind="ExternalInput").ap()
    # cvec columns: 0/1 = bq per o-tile, 2/3 = bk per o-tile, 4/5 = bp_eff per
    # c-tile.  (The 1/C and ones constant blocks are memset on-device.)
    cvec_d = nc.dram_tensor("cvec", [128, 6], f32, kind="ExternalInput").ap()
    out_d = nc.dram_tensor("out", [C, N], f32, kind="ExternalOutput").ap()

    # m-tiles covered by each chunk: chunk j covers m in [off/128, (off+w)/128)
    def chunk_mtiles(ji):
        off, w = CHUNKS[ji]
        return range(off // 128, (off + w) // 128)

    with tile.TileContext(nc) as tc, ExitStack() as ctx:
        persist = ctx.enter_context(tc.tile_pool(name="persist", bufs=1))

        # DMA notes: (1) each descriptor covers one partition-row segment and
        # the queues are descriptor-rate-bound (~50ns each), so loads are
        # split into partition strips (full rows = max bytes per descriptor);
        # (2) each dma_start costs ~600ns of *issue* time on its engine's
        # sequencer, so issues are spread across the four idle sequencers.
        def strip_load(eng, dst, src, prows):
            P = dst.shape[0]
            for p in range(0, P, prows):
                pe = min(P, p + prows)
                eng.dma_start(dst[p:pe, :], src[p:pe, :])

        # ---- constants built on-device (no DMA) ------------------------
        invC = persist.tile([128, 128], bf16, tag="invC", name="invC")
        nc.vector.memset(invC[:], 1.0 / C)
        onesb = persist.tile([128, 128], bf16, tag="onesb", name="onesb")
        nc.gpsimd.memset(onesb[:], 1.0)

        # ---- x2 first: it gates k/v -> attention -----------------------
        x2sc = ctx.enter_context(tc.tile_pool(name="x2scope", bufs=1))
        x2_t = [
            x2sc.tile([128, N], bf16, tag=f"x2_{ct}", name=f"x2_{ct}")
            for ct in range(CT)
        ]
        strip_load(nc.sync, x2_t[0], x2_d[0:128, :], 64)
        strip_load(nc.scalar, x2_t[1], x2_d[128:256, :], 64)

        w_tiles = {}
        for (nm, d), eng in ((("k", wkt_d), nc.sync), (("v", wvt_d), nc.scalar)):
            for ct in range(CT):
                t = persist.tile([128, C], bf16, tag=f"w{nm}{ct}", name=f"w{nm}{ct}")
                strip_load(eng, t, d[ct * 128 : (ct + 1) * 128, :], 64)
                w_tiles[(nm, ct)] = t
        cvec = persist.tile([128, 6], f32, tag="cvec", name="cvec")
        nc.sync.dma_start(cvec[:], cvec_d[:, :])

        x1_t = [
            persist.tile([128, N], bf16, tag=f"x1_{ct}", name=f"x1_{ct}")
            for ct in range(CT)
        ]
        strip_load(nc.sync, x1_t[0], x1_d[0:128, :], 64)
        strip_load(nc.scalar, x1_t[1], x1_d[128:256, :], 64)

        for nm, d in (("q", wqt_d), ("p", wpt_d)):
            for ct in range(CT):
                t = persist.tile([128, C], bf16, tag=f"w{nm}{ct}", name=f"w{nm}{ct}")
                nc.gpsimd.dma_start(t[:], d[ct * 128 : (ct + 1) * 128, :])
                w_tiles[(nm, ct)] = t

        # persistent intermediates
        k_t = [
            persist.tile([128, N], bf16, tag=f"k{ot}", name=f"k{ot}")
            for ot in range(CT)
        ]
        vT_t = [
            persist.tile([128, C], bf16, tag=f"vT{m}", name=f"vT{m}")
            for m in range(MT)
        ]
        xn1_t = [
            persist.tile([128, N], bf16, tag=f"xn1_{ct}", name=f"xn1_{ct}")
            for ct in range(CT)
        ]

        # ------------------------------------------------------------------
        # Pre-phase: per-chunk pipeline  stats -> xn -> k/vT   (x2 stream)
        # plus the x1 stats/xn stream (feeds q projections later).
        # ------------------------------------------------------------------
        with (
            tc.tile_pool(name="scr", bufs=3) as scr,
            tc.tile_pool(name="xnp", bufs=6) as xnp,
            tc.tile_pool(name="ps_st", bufs=2, space="PSUM") as ps_st,
            tc.tile_pool(name="ps_kv", bufs=2, space="PSUM") as ps_kv,
        ):

            def emit_stats_xn(tsel, ji, xsrc, xn_out):
                """stats + xn for (tensor tsel, chunk ji).

                xsrc: list of [128, N] bf16 tiles (per ct)
                xn_out: dict key (ct) -> (tile, col_off) destination slices
                """
                off, w = CHUNKS[ji]
                ub = ps_st.tile([128, 512], f32, tag="ub", name="ub")
                for ct in range(CT):
                    nc.tensor.matmul(
                        ub[:, :w],
                        invC[:],
                        xsrc[ct][:, off : off + w],
                        start=(ct == 0),
                        stop=(ct == CT - 1),
                    )
                ms = ps_st.tile([128, 512], f32, tag="ms", name="ms")
                for ct in range(CT):
                    xsq = scr.tile([128, 512], bf16, tag="xsq", name="xsq")
                    nc.gpsimd.tensor_mul(
                        xsq[:, :w],
                        xsrc[ct][:, off : off + w],
                        xsrc[ct][:, off : off + w],
                    )
                    nc.tensor.matmul(
                        ms[:, :w],
                        invC[:],
                        xsq[:, :w],
                        start=(ct == 0),
                        stop=(ct == CT - 1),
                    )
                usq = scr.tile([128, 512], f32, tag="usq", name="usq")
                nc.scalar.square(usq[:, :w], ub[:, :w])
                var = scr.tile([128, 512], f32, tag="var", name="var")
                nc.vector.scalar_tensor_tensor(
                    var[:, :w], ms[:, :w], EPS, usq[:, :w], ADD, SUB
                )
                std = scr.tile([128, 512], f32, tag="std", name="std")
                nc.scalar.activation(
                    std[:, :w], var[:, :w], mybir.ActivationFunctionType.Sqrt
                )
                rstd = scr.tile([128, 512], f32, tag=f"rstd{tsel}", name=f"rstd{tsel}")
                nc.vector.reciprocal_approx_fast(rstd[:, :w], std[:, :w])
                for ct in range(CT):
                    d = scr.tile([128, 512], f32, tag="xnd", name="xnd")
                    nc.vector.tensor_sub(
                        d[:, :w], xsrc[ct][:, off : off + w], ub[:, :w]
                    )
                    dst, dcol = xn_out[ct]
                    nc.vector.tensor_mul(
                        dst[:, dcol : dcol + w], d[:, :w], rstd[:, :w]
                    )

            xn2 = {}

            def emit_kv(ji):
                off, w = CHUNKS[ji]
                # k projection for this chunk of tokens
                for ot in range(CT):
                    ps = ps_kv.tile([128, 512], f32, tag="kv", name="kv")
                    for ct in range(CT):
                        nc.tensor.matmul(
                            ps[:, :w],
                            w_tiles[("k", ct)][:, ot * 128 : (ot + 1) * 128],
                            xn2[(ji, ct)][:, :w],
                            start=(ct == 0),
                            stop=(ct == CT - 1),
                        )
                    nc.vector.tensor_scalar_add(
                        k_t[ot][:, off : off + w], ps[:, :w], cvec[:, 2 + ot : 3 + ot]
                    )
                # vT for the m-tiles inside this chunk
                for m in chunk_mtiles(ji):
                    coff = m * 128 - off
                    ps = ps_kv.tile([128, C], f32, tag="kv", name="kv")
                    for ct in range(CT):
                        nc.tensor.matmul(
                            ps[:],
                            xn2[(ji, ct)][:, coff : coff + 128],
                            w_tiles[("v", ct)][:, :],
                            start=(ct == 0),
                            stop=(ct == CT - 1),
                        )
                    nc.scalar.copy(vT_t[m][:], ps[:])

            # x2 chunk 0 first (longest chain), then interleave x1 stats so
            # the PE always has stats matmuls to chew on while DVE chains run.
            for ji in range(NJ):
                for ct in range(CT):
                    t = xnp.tile([128, 512], bf16, tag="xn2", name=f"xn2_{ji}_{ct}")
                    xn2[(ji, ct)] = t
                emit_stats_xn(1, ji, x2_t, {ct: (xn2[(ji, ct)], 0) for ct in range(CT)})
                emit_kv(ji)
                # x1 stream trails: its xn goes to the persistent xn1 tiles
                emit_stats_xn(
                    0, ji, x1_t, {ct: (xn1_t[ct], CHUNKS[ji][0]) for ct in range(CT)}
                )

        # ------------------------------------------------------------------
        # Attention: per q-chunk; q projected one chunk ahead; epilogue
        # (normalize, Wp projection, residual, DMA out) inside the loop.
        # ------------------------------------------------------------------
        with (
            tc.tile_pool(name="qch", bufs=4) as qch,
            tc.tile_pool(name="pt", bufs=24) as pt_pool,
            tc.tile_pool(name="oup", bufs=4) as oup,
            tc.tile_pool(name="invp", bufs=2) as invp,
            tc.tile_pool(name="outp", bufs=4) as outp,
            tc.tile_pool(name="ps_pj", bufs=2, space="PSUM") as ps_pj,
            tc.tile_pool(name="ps_qk", bufs=2, space="PSUM") as ps_qk,
            tc.tile_pool(name="ps_o", bufs=3, space="PSUM") as ps_o,
            tc.tile_pool(name="ps_rs", bufs=1, space="PSUM") as ps_rs,
        ):
            q_ch = {}

            def emit_qproj(ji):
                off, w = CHUNKS[ji]
                for ot in range(CT):
                    ps = ps_pj.tile([128, 512], f32, tag="pj", name="pj")
                    for ct in range(CT):
                        nc.tensor.matmul(
                            ps[:, :w],
                            w_tiles[("q", ct)][:, ot * 128 : (ot + 1) * 128],
                            xn1_t[ct][:, off : off + w],
                            start=(ct == 0),
                            stop=(ct == CT - 1),
                        )
                    qt = qch.tile([128, 512], bf16, tag="q", name=f"q{ji}_{ot}")
                    nc.vector.tensor_scalar_add(
                        qt[:, :w], ps[:, :w], cvec[:, 0 + ot : 1 + ot]
                    )
                    q_ch[(ji, ot)] = qt

            emit_qproj(0)
            for ji, (off, w) in enumerate(CHUNKS):
                if ji + 1 < NJ:
                    emit_qproj(ji + 1)
                st = {}

                def emit_qk(m):
                    ps = ps_qk.tile([128, 512], f32, tag="st", name="st")
                    for ot in range(CT):
                        nc.tensor.matmul(
                            ps[:, :w],
                            k_t[ot][:, m * 128 : (m + 1) * 128],
                            q_ch[(ji, ot)][:, :w],
                            start=(ot == 0),
                            stop=(ot == CT - 1),
                        )
                    st[m] = ps

                o_ps = [
                    ps_o.tile([128, 512], f32, tag="o", name="o") for _ in range(CT)
                ]

                pts = []
                emit_qk(0)
                for m in range(MT):
                    if m + 1 < MT:
                        emit_qk(m + 1)
                    pt = pt_pool.tile([128, 512], bf16, tag="pt", name=f"pt{m}")
                    nc.scalar.activation(
                        pt[:, :w], st[m][:, :w], mybir.ActivationFunctionType.Exp
                    )
                    del st[m]
                    pts.append(pt)
                    for c in range(CT):
                        nc.tensor.matmul(
                            o_ps[c][:, :w],
                            vT_t[m][:, c * 128 : (c + 1) * 128],
                            pt[:, :w],
                            start=(m == 0),
                            stop=(m == MT - 1),
                        )

                # ---- chunk epilogue -----------------------------------
                # rowsum as one end-of-chunk block of M=128 ones-matmuls:
                # the same lhsT back-to-back pipelines weight loads, and the
                # [128, w] result is already broadcast across partitions so
                # the reciprocal consumes the PSUM directly.
                rs_ps = ps_rs.tile([128, 512], f32, tag="rsp", name="rsp")
                for m in range(MT):
                    nc.tensor.matmul(
                        rs_ps[:, :w],
                        onesb[:, 0:128],
                        pts[m][:, :w],
                        start=(m == 0),
                        stop=(m == MT - 1),
                    )
                inv_b = invp.tile([128, 512], f32, tag="invb", name="invb")
                nc.vector.reciprocal_approx_fast(inv_b[:, :w], rs_ps[:, :w])

                ou = []
                for c in range(CT):
                    t = oup.tile([128, 512], bf16, tag="ou", name=f"ou{c}")
                    nc.vector.tensor_mul(t[:, :w], o_ps[c][:, :w], inv_b[:, :w])
                    ou.append(t)

                for ct in range(CT):
                    ps = ps_pj.tile([128, 512], f32, tag="pj", name="pj")
                    for ci in range(CT):
                        nc.tensor.matmul(
                            ps[:, :w],
                            w_tiles[("p", ci)][:, ct * 128 : (ct + 1) * 128],
                            ou[ci][:, :w],
                            start=(ci == 0),
                            stop=(ci == CT - 1),
                        )
                    ot_t = outp.tile([128, 512], f32, tag="outt", name=f"out{ct}")
                    nc.vector.scalar_tensor_tensor(
                        ot_t[:, :w],
                        ps[:, :w],
                        cvec[:, 4 + ct : 5 + ct],
                        x1_t[ct][:, off : off + w],
                        ADD,
                        ADD,
                    )
                    if ji + 1 < NJ:
                        nc.sync.dma_start(
                            out_d[ct * 128 : (ct + 1) * 128, off : off + w],
                            ot_t[:, :w],
                        )
                    else:
                        # last chunk is latency-critical: strip across queues
                        for p in range(0, 128, 64):
                            nc.sync.dma_start(
                                out_d[ct * 128 + p : ct * 128 + p + 64, off : off + w],
                                ot_t[p : p + 64, :w],
                            )

    nc.compile()
    return nc


def _host_prep(inputs):
    f = lambda k: np.asarray(inputs[k], dtype=np.float32)
    Wq, Wk, Wv, Wp = f("Wq"), f("Wk"), f("Wv"), f("Wp")
    bq, bk, bv, bp = f("bq"), f("bk"), f("bv"), f("bp")
    w_nq, b_nq, w_nkv, b_nkv = f("w_nq"), f("b_nq"), f("w_nkv"), f("b_nkv")

    Wq_eff = Wq * w_nq[None, :] * SCALE
    bq_eff = SCALE * (bq + Wq @ b_nq)
    Wk_eff = Wk * w_nkv[None, :]
    bk_eff = bk + Wk @ b_nkv
    Wv_eff = Wv * w_nkv[None, :]
    bv_eff = bv + Wv @ b_nkv
    bp_eff = bp + Wp @ bv_eff  # v bias folded through softmax + Wp

    wqt = np.ascontiguousarray(Wq_eff.T).astype(BF16)
    wkt = np.ascontiguousarray(Wk_eff.T).astype(BF16)
    wvt = np.ascontiguousarray(Wv_eff.T).astype(BF16)
    wpt = np.ascontiguousarray(Wp.T).astype(BF16)

    cvec = np.zeros((128, 6), np.float32)
    cvec[:, 0] = bq_eff[0:128]
    cvec[:, 1] = bq_eff[128:256]
    cvec[:, 2] = bk_eff[0:128]
    cvec[:, 3] = bk_eff[128:256]
    cvec[:, 4] = bp_eff[0:128]
    cvec[:, 5] = bp_eff[128:256]

    return dict(wqt=wqt, wkt=wkt, wvt=wvt, wpt=wpt, cvec=cvec)


def _maybe_patch_ldw_opt():
    if os.environ.get("BASS_LDW_OPT", "0") != "1":
        return
    import concourse.bass_utils as bu
    if getattr(bu, "_ldw_patch", False):
        return
    orig = bu.run_command
    def patched(argv, **kw):
        if isinstance(argv, list):
            argv = [a.replace("--enable-ldw-opt=false", "--enable-ldw-opt=true") for a in argv]
        return orig(argv, **kw)
    bu.run_command = patched
    bu._ldw_patch = True


def kernel(**inputs):
    global last_results
    _maybe_patch_ldw_opt()
    from concourse.bass_utils import run_bass_kernel_spmd

    if "nc" not in _cache:
        _cache["nc"] = _build_program()
    nc = _cache["nc"]

    shared = _host_prep(inputs)
    x1 = np.asarray(inputs["x1"], dtype=np.float32).reshape(B, C, N).astype(BF16)
    x2 = np.asarray(inputs["x2"], dtype=np.float32).reshape(B, C, N).astype(BF16)

    in_maps = []
    for b in range(B):
        m = dict(shared)
        m["x1"] = np.ascontiguousarray(x1[b])
        m["x2"] = np.ascontiguousarray(x2[b])
        in_maps.append(m)

    trace = os.environ.get("BASS_KERNEL_TRACE", "0") == "1"
    res = run_bass_kernel_spmd(
        nc, in_maps, core_ids=list(range(B)), trace=trace
    )
    last_results = res
    out = np.stack([res.results[b]["out"].reshape(C, H, W) for b in range(B)])
    return out.astype(np.float32)


# revision 33
# speedup vs baseline: 1.3042x; 1.0384x over previous
"""Trainium2 Bass kernel for nn_CrossAttentionBlock (B=8, C=256, H=W=48).

Sharding: data-parallel over batch B — one batch per NeuronCore (8 cores).

Per-core math (x: [C=256, N=2304] f32):
  LayerNorm over C folded into projection weights on host:
      W_eff = W * w_n[None,:],  b_eff = b + W @ b_n
  attention SCALE folded into Wq_eff / bq_eff.
  v bias folded into the output bias (softmax rows sum to one, so
  attn@(v+bv) = attn@v + bv, hence bp_eff = bp + Wp@bv_eff).

  Activations are host-cast to bf16 and DMA'd as [32, N] partition strips
  (the DMA queues are descriptor-rate-bound; full-row descriptors maximize
  bytes per descriptor).  Stats: mean via a 1/C-ones bf16 matmul on x;
  mean-square via a Pool-computed x*x fed to a second ones-matmul.
  xn = (x-u)*rstd on the DVE, emitted as bf16.

  Attention is computed transposed:  St[m,n] = sum_o k[o,m] q[o,n]
  so softmax normalization runs over the *partition* axis m:
    - no row-max subtraction (logits bounded ~21, exp safe in f32)
    - P = exp(St) (ScalarE, PSUM->SBUF bf16 eviction)
    - rowsum[n] = sum_m P[m,n] via an M=1 ones-matmul accumulated across m
    - 1/rowsum applied AFTER the output projection (scaling commutes
      with Wp); the reciprocal row is partition-broadcast by the Pool
      engine and fused into the PSUM->SBUF eviction of the attention
      output, and the Wp projection + residual + DMA-out run per chunk
      inside the attention loop so there is no serial tail.
  v is produced directly transposed (vT[m,o] = sum_c xn2[c,m] WvT[c,o])
  so P.V contracts over m on partitions with zero PE transposes.
"""

import os
import sys
import types
import ctypes
import contextlib

sys.path.insert(0, "/opt/trn_rl_repo")

import numpy as np
import ml_dtypes

# ---------------------------------------------------------------------------
# NTFF profile hook stub (antenv.axon_hooks is absent in this container; the
# ctypes shim mirrors trn_agent_boot). Only used when tracing is requested.
# ---------------------------------------------------------------------------


def _ntff_profile_via_ctypes(so_path):
    try:
        lib = ctypes.CDLL(so_path)
    except OSError:
        return None
    if not hasattr(lib, "axon_start_nrt_profile"):
        return None
    lib.axon_start_nrt_profile.argtypes = [
        ctypes.POINTER(ctypes.c_int64),
        ctypes.c_size_t,
    ]
    lib.axon_start_nrt_profile.restype = ctypes.c_int64
    lib.axon_stop_nrt_profile.argtypes = [ctypes.c_char_p]
    lib.axon_stop_nrt_profile.restype = ctypes.c_int64

    @contextlib.contextmanager
    def _hook(output_dir, device_ids):
        import jax

        jax.devices()
        if device_ids:
            ids = (ctypes.c_int64 * len(device_ids))(*device_ids)
            rc = lib.axon_start_nrt_profile(ids, len(device_ids))
        else:
            rc = lib.axon_start_nrt_profile(None, 0)
        if rc != 0:
            raise RuntimeError(f"axon_start_nrt_profile rc={rc}")
        try:
            yield
        finally:
            n = lib.axon_stop_nrt_profile(str(output_dir).encode())
            print(f"profile: {n} file(s) written to {output_dir}", file=sys.stderr)

    return _hook


if "antenv.axon_hooks" not in sys.modules:
    _hook = _ntff_profile_via_ctypes("/opt/axon/libaxon_pjrt.so")
    _mod = types.ModuleType("antenv.axon_hooks")
    _mod.get_axon_ntff_profile_hook = lambda: _hook
    sys.modules["antenv.axon_hooks"] = _mod

# ---------------------------------------------------------------------------

B, C, H, W = 8, 256, 48, 48
N = H * W  # 2304
SCALE = (C // 8) ** (-0.5)
EPS = 1e-6
CT = C // 128  # 2 channel tiles
MT = N // 128  # 18 m (key-token) tiles
CHUNKS = [(0, 512), (512, 512), (1024, 512), (1536, 512), (2048, 256)]
NJ = len(CHUNKS)

BF16 = ml_dtypes.bfloat16

_cache = {}
last_results = None  # BassKernelResults of the most recent run (for test.py)


def _build_program():
    import concourse.bacc as bacc
    import concourse.tile as tile
    import concourse.mybir as mybir
    from contextlib import ExitStack

    f32 = mybir.dt.float32
    bf16 = mybir.dt.bfloat16
    ADD = mybir.AluOpType.add
    SUB = mybir.AluOpType.subtract

    nc = bacc.Bacc("TRN2", target_bir_lowering=False, debug=False)

    x1_d = nc.dram_tensor("x1", [C, N], bf16, kind="ExternalInput").ap()
    x2_d = nc.dram_tensor("x2", [C, N], bf16, kind="ExternalInput").ap()
    wqt_d = nc.dram_tensor("wqt", [C, C], bf16, kind="ExternalInput").ap()
    wkt_d = nc.dram_tensor("wkt", [C, C], bf16, kind="ExternalInput").ap()
    wvt_d = nc.dram_tensor("wvt", [C, C], bf16, kind="ExternalInput").ap()
    wpt_d = nc.dram_tensor("wpt", [C, C], bf16, kind="ExternalInput").ap()
    # cvec columns: 0/1 = bq per o-tile, 2/3 = bk per o-tile, 4/5 = bp_eff per
    # c-tile.  (The 1/C and ones constant blocks are memset on-device.)
    cvec_d = nc.dram_tensor("cvec", [128, 6], f32, kind="ExternalInput").ap()
    out_d = nc.dram_tensor("out", [C, N], f32, kind="ExternalOutput").ap()

    # m-tiles covered by each chunk: chunk j covers m in [off/128, (off+w)/128)
    def chunk_mtiles(ji):
        off, w = CHUNKS[ji]
        return range(off // 128, (off + w) // 128)

    with tile.TileContext(nc) as tc, ExitStack() as ctx:
        persist = ctx.enter_context(tc.tile_pool(name="persist", bufs=1))

        # DMA notes: (1) each descriptor covers one partition-row segment and
        # the queues are descriptor-rate-bound (~50ns each), so loads are
        # split into partition strips (full rows = max bytes per descriptor);
        # (2) each dma_start costs ~600ns of *issue* time on its engine's
        # sequencer, so issues are spread across the four idle sequencers.
        def strip_load(eng, dst, src, prows):
            P = dst.shape[0]
            for p in range(0, P, prows):
                pe = min(P, p + prows)
                eng.dma_start(dst[p:pe, :], src[p:pe, :])

        # ---- constants built on-device (no DMA) ------------------------
        invC = persist.tile([128, 128], bf16, tag="invC", name="invC")
        nc.vector.memset(invC[:], 1.0 / C)
        onesb = persist.tile([128, 128], bf16, tag="onesb", name="onesb")
        nc.gpsimd.memset(onesb[:], 1.0)

        # ---- x2 first: it gates k/v -> attention -----------------------
        x2sc = ctx.enter_context(tc.tile_pool(name="x2scope", bufs=1))
        x2_t = [
            x2sc.tile([128, N], bf16, tag=f"x2_{ct}", name=f"x2_{ct}")
            for ct in range(CT)
        ]
        strip_load(nc.sync, x2_t[0], x2_d[0:128, :], 64)
        strip_load(nc.scalar, x2_t[1], x2_d[128:256, :], 64)

        w_tiles = {}
        for (nm, d), eng in ((("k", wkt_d), nc.sync), (("v", wvt_d), nc.scalar)):
            for ct in range(CT):
                t = persist.tile([128, C], bf16, tag=f"w{nm}{ct}", name=f"w{nm}{ct}")
                strip_load(eng, t, d[ct * 128 : (ct + 1) * 128, :], 64)
                w_tiles[(nm, ct)] = t
        cvec = persist.tile([128, 6], f32, tag="cvec", name="cvec")
        nc.sync.dma_start(cvec[:], cvec_d[:, :])

        x1_t = [
            persist.tile([128, N], bf16, tag=f"x1_{ct}", name=f"x1_{ct}")
            for ct in range(CT)
        ]
        strip_load(nc.sync, x1_t[0], x1_d[0:128, :], 64)
        strip_load(nc.scalar, x1_t[1], x1_d[128:256, :], 64)

        for nm, d in (("q", wqt_d), ("p", wpt_d)):
            for ct in range(CT):
                t = persist.tile([128, C], bf16, tag=f"w{nm}{ct}", name=f"w{nm}{ct}")
                nc.gpsimd.dma_start(t[:], d[ct * 128 : (ct + 1) * 128, :])
                w_tiles[(nm, ct)] = t

        # persistent intermediates
        k_t = [
            persist.tile([128, N], bf16, tag=f"k{ot}", name=f"k{ot}")
            for ot in range(CT)
        ]
        vT_t = [
            persist.tile([128, C], bf16, tag=f"vT{m}", name=f"vT{m}")
            for m in range(MT)
        ]
        xn1_t = [
            persist.tile([128, N], bf16, tag=f"xn1_{ct}", name=f"xn1_{ct}")
            for ct in range(CT)
        ]

        # ------------------------------------------------------------------
        # Pre-phase: per-chunk pipeline  stats -> xn -> k/vT   (x2 stream)
        # plus the x1 stats/xn stream (feeds q projections later).
        # ------------------------------------------------------------------
        with (
            tc.tile_pool(name="scr", bufs=3) as scr,
            tc.tile_pool(name="xnp", bufs=6) as xnp,
            tc.tile_pool(name="ps_st", bufs=2, space="PSUM") as ps_st,
            tc.tile_pool(name="ps_kv", bufs=2, space="PSUM") as ps_kv,
        ):

            def emit_stats_xn(tsel, ji, xsrc, xn_out):
                """stats + xn for (tensor tsel, chunk ji).

                xsrc: list of [128, N] bf16 tiles (per ct)
                xn_out: dict key (ct) -> (tile, col_off) destination slices
                """
                off, w = CHUNKS[ji]
                ub = ps_st.tile([128, 512], f32, tag="ub", name="ub")
                for ct in range(CT):
                    nc.tensor.matmul(
                        ub[:, :w],
                        invC[:],
                        xsrc[ct][:, off : off + w],
                        start=(ct == 0),
                        stop=(ct == CT - 1),
                    )
                ms = ps_st.tile([128, 512], f32, tag="ms", name="ms")
                for ct in range(CT):
                    xsq = scr.tile([128, 512], bf16, tag="xsq", name="xsq")
                    nc.gpsimd.tensor_mul(
                        xsq[:, :w],
                        xsrc[ct][:, off : off + w],
                        xsrc[ct][:, off : off + w],
                    )
                    nc.tensor.matmul(
                        ms[:, :w],
                        invC[:],
                        xsq[:, :w],
                        start=(ct == 0),
                        stop=(ct == CT - 1),
                    )
                usq = scr.tile([128, 512], f32, tag="usq", name="usq")
                nc.scalar.square(usq[:, :w], ub[:, :w])
                var = scr.tile([128, 512], f32, tag="var", name="var")
                nc.vector.scalar_tensor_tensor(
                    var[:, :w], ms[:, :w], EPS, usq[:, :w], ADD, SUB
                )
                std = scr.tile([128, 512], f32, tag="std", name="std")
                nc.scalar.activation(
                    std[:, :w], var[:, :w], mybir.ActivationFunctionType.Sqrt
                )
                rstd = scr.tile([128, 512], f32, tag=f"rstd{tsel}", name=f"rstd{tsel}")
                nc.vector.reciprocal_approx_fast(rstd[:, :w], std[:, :w])
                for ct in range(CT):
                    d = scr.tile([128, 512], f32, tag="xnd", name="xnd")
                    nc.vector.tensor_sub(
                        d[:, :w], xsrc[ct][:, off : off + w], ub[:, :w]
                    )
                    dst, dcol = xn_out[ct]
                    # x1's multiplies go to the otherwise-idle Pool engine
                    eng = nc.vector if tsel == 1 else nc.gpsimd
                    eng.tensor_mul(dst[:, dcol : dcol + w], d[:, :w], rstd[:, :w])

            xn2 = {}

            def emit_kv(ji):
                off, w = CHUNKS[ji]
                # k projection for this chunk of tokens
                for ot in range(CT):
                    ps = ps_kv.tile([128, 512], f32, tag="kv", name="kv")
                    for ct in range(CT):
                        nc.tensor.matmul(
                            ps[:, :w],
                            w_tiles[("k", ct)][:, ot * 128 : (ot + 1) * 128],
                            xn2[(ji, ct)][:, :w],
                            start=(ct == 0),
                            stop=(ct == CT - 1),
                        )
                    nc.vector.tensor_scalar_add(
                        k_t[ot][:, off : off + w], ps[:, :w], cvec[:, 2 + ot : 3 + ot]
                    )
                # vT for the m-tiles inside this chunk
                for m in chunk_mtiles(ji):
                    coff = m * 128 - off
                    ps = ps_kv.tile([128, C], f32, tag="kv", name="kv")
                    for ct in range(CT):
                        nc.tensor.matmul(
                            ps[:],
                            xn2[(ji, ct)][:, coff : coff + 128],
                            w_tiles[("v", ct)][:, :],
                            start=(ct == 0),
                            stop=(ct == CT - 1),
                        )
                    nc.scalar.copy(vT_t[m][:], ps[:])

            # x2 stream first (it gates the attention m-loop chunk by chunk);
            # x1 chunk 0 next (it gates qproj(0) and thus attention start);
            # x1 chunks 1-4 are demoted BELOW the attention loop's priority
            # so they only fill engine bubbles during attention.
            for ji in range(NJ):
                for ct in range(CT):
                    t = xnp.tile([128, 512], bf16, tag="xn2", name=f"xn2_{ji}_{ct}")
                    xn2[(ji, ct)] = t
                emit_stats_xn(1, ji, x2_t, {ct: (xn2[(ji, ct)], 0) for ct in range(CT)})
                emit_kv(ji)
                if ji == 0:
                    emit_stats_xn(
                        0, 0, x1_t, {ct: (xn1_t[ct], 0) for ct in range(CT)}
                    )
            with tc.high_priority(offset=-(10**6)):
                for ji in range(1, NJ):
                    emit_stats_xn(
                        0, ji, x1_t,
                        {ct: (xn1_t[ct], CHUNKS[ji][0]) for ct in range(CT)},
                    )

        # ------------------------------------------------------------------
        # Attention: per q-chunk; q projected one chunk ahead; epilogue
        # (normalize, Wp projection, residual, DMA out) inside the loop.
        # ------------------------------------------------------------------
        with (
            tc.tile_pool(name="qch", bufs=4) as qch,
            tc.tile_pool(name="pt", bufs=24) as pt_pool,
            tc.tile_pool(name="oup", bufs=4) as oup,
            tc.tile_pool(name="invp", bufs=2) as invp,
            tc.tile_pool(name="outp", bufs=4) as outp,
            tc.tile_pool(name="ps_pj", bufs=2, space="PSUM") as ps_pj,
            tc.tile_pool(name="ps_qk", bufs=2, space="PSUM") as ps_qk,
            tc.tile_pool(name="ps_o", bufs=3, space="PSUM") as ps_o,
            tc.tile_pool(name="ps_rs", bufs=1, space="PSUM") as ps_rs,
        ):
            q_ch = {}

            def emit_qproj(ji):
                off, w = CHUNKS[ji]
                for ot in range(CT):
                    ps = ps_pj.tile([128, 512], f32, tag="pj", name="pj")
                    for ct in range(CT):
                        nc.tensor.matmul(
                            ps[:, :w],
                            w_tiles[("q", ct)][:, ot * 128 : (ot + 1) * 128],
                            xn1_t[ct][:, off : off + w],
                            start=(ct == 0),
                            stop=(ct == CT - 1),
                        )
                    qt = qch.tile([128, 512], bf16, tag="q", name=f"q{ji}_{ot}")
                    nc.vector.tensor_scalar_add(
                        qt[:, :w], ps[:, :w], cvec[:, 0 + ot : 1 + ot]
                    )
                    q_ch[(ji, ot)] = qt

            emit_qproj(0)
            for ji, (off, w) in enumerate(CHUNKS):
                if ji + 1 < NJ:
                    emit_qproj(ji + 1)
                st = {}

                def emit_qk(m):
                    ps = ps_qk.tile([128, 512], f32, tag="st", name="st")
                    for ot in range(CT):
                        nc.tensor.matmul(
                            ps[:, :w],
                            k_t[ot][:, m * 128 : (m + 1) * 128],
                            q_ch[(ji, ot)][:, :w],
                            start=(ot == 0),
                            stop=(ot == CT - 1),
                        )
                    st[m] = ps

                o_ps = [
                    ps_o.tile([128, 512], f32, tag="o", name="o") for _ in range(CT)
                ]

                # Last chunk: interleave the rowsum accumulation into the
                # m-loop so nothing serializes after the final PV (tail).
                inline_rs = ji == NJ - 1
                rs_ps = ps_rs.tile([128, 512], f32, tag="rsp", name="rsp")

                pts = []
                emit_qk(0)
                for m in range(MT):
                    if m + 1 < MT:
                        emit_qk(m + 1)
                    pt = pt_pool.tile([128, 512], bf16, tag="pt", name=f"pt{m}")
                    nc.scalar.activation(
                        pt[:, :w], st[m][:, :w], mybir.ActivationFunctionType.Exp
                    )
                    del st[m]
                    pts.append(pt)
                    for c in range(CT):
                        nc.tensor.matmul(
                            o_ps[c][:, :w],
                            vT_t[m][:, c * 128 : (c + 1) * 128],
                            pt[:, :w],
                            start=(m == 0),
                            stop=(m == MT - 1),
                        )
                    if inline_rs:
                        nc.tensor.matmul(
                            rs_ps[:, :w],
                            onesb[:, 0:128],
                            pt[:, :w],
                            start=(m == 0),
                            stop=(m == MT - 1),
                        )

                # ---- chunk epilogue -----------------------------------
                # rowsum as one end-of-chunk block of M=128 ones-matmuls:
                # the same lhsT back-to-back pipelines weight loads, and the
                # [128, w] result is already broadcast across partitions so
                # the reciprocal consumes the PSUM directly.
                if not inline_rs:
                    for m in range(MT):
                        nc.tensor.matmul(
                            rs_ps[:, :w],
                            onesb[:, 0:128],
                            pts[m][:, :w],
                            start=(m == 0),
                            stop=(m == MT - 1),
                        )
                inv_b = invp.tile([128, 512], f32, tag="invb", name="invb")
                nc.vector.reciprocal_approx_fast(inv_b[:, :w], rs_ps[:, :w])

                ou = []
                for c in range(CT):
                    t = oup.tile([128, 512], bf16, tag="ou", name=f"ou{c}")
                    nc.vector.tensor_mul(t[:, :w], o_ps[c][:, :w], inv_b[:, :w])
                    ou.append(t)

                for ct in range(CT):
                    ps = ps_pj.tile([128, 512], f32, tag="pj", name="pj")
                    for ci in range(CT):
                        nc.tensor.matmul(
                            ps[:, :w],
                            w_tiles[("p", ci)][:, ct * 128 : (ct + 1) * 128],
                            ou[ci][:, :w],
                            start=(ci == 0),
                            stop=(ci == CT - 1),
                        )
                    ot_t = outp.tile([128, 512], f32, tag="outt", name=f"out{ct}")
                    nc.vector.scalar_tensor_tensor(
                        ot_t[:, :w],
                        ps[:, :w],
                        cvec[:, 4 + ct : 5 + ct],
                        x1_t[ct][:, off : off + w],
                        ADD,
                        ADD,
                    )
                    if ji + 1 < NJ:
                        nc.sync.dma_start(
                            out_d[ct * 128 : (ct + 1) * 128, off : off + w],
                            ot_t[:, :w],
                        )
                    else:
                        # last chunk is latency-critical: strip across queues
                        # and split the issue cost across two sequencers
                        eng = nc.sync if ct == 0 else nc.scalar
                        for p in range(0, 128, 64):
                            eng.dma_start(
                                out_d[ct * 128 + p : ct * 128 + p + 64, off : off + w],
                                ot_t[p : p + 64, :w],
                            )

    nc.compile()
    return nc


def _host_prep(inputs):
    f = lambda k: np.asarray(inputs[k], dtype=np.float32)
    Wq, Wk, Wv, Wp = f("Wq"), f("Wk"), f("Wv"), f("Wp")
    bq, bk, bv, bp = f("bq"), f("bk"), f("bv"), f("bp")
    w_nq, b_nq, w_nkv, b_nkv = f("w_nq"), f("b_nq"), f("w_nkv"), f("b_nkv")

    Wq_eff = Wq * w_nq[None, :] * SCALE
    bq_eff = SCALE * (bq + Wq @ b_nq)
    Wk_eff = Wk * w_nkv[None, :]
    bk_eff = bk + Wk @ b_nkv
    Wv_eff = Wv * w_nkv[None, :]
    bv_eff = bv + Wv @ b_nkv
    bp_eff = bp + Wp @ bv_eff  # v bias folded through softmax + Wp

    wqt = np.ascontiguousarray(Wq_eff.T).astype(BF16)
    wkt = np.ascontiguousarray(Wk_eff.T).astype(BF16)
    wvt = np.ascontiguousarray(Wv_eff.T).astype(BF16)
    wpt = np.ascontiguousarray(Wp.T).astype(BF16)

    cvec = np.zeros((128, 6), np.float32)
    cvec[:, 0] = bq_eff[0:128]
    cvec[:, 1] = bq_eff[128:256]
    cvec[:, 2] = bk_eff[0:128]
    cvec[:, 3] = bk_eff[128:256]
    cvec[:, 4] = bp_eff[0:128]
    cvec[:, 5] = bp_eff[128:256]

    return dict(wqt=wqt, wkt=wkt, wvt=wvt, wpt=wpt, cvec=cvec)


def _maybe_patch_ldw_opt():
    if os.environ.get("BASS_LDW_OPT", "0") != "1":
        return
    import concourse.bass_utils as bu
    if getattr(bu, "_ldw_patch", False):
        return
    orig = bu.run_command
    def patched(argv, **kw):
        if isinstance(argv, list):
            argv = [a.replace("--enable-ldw-opt=false", "--enable-ldw-opt=true") for a in argv]
        return orig(argv, **kw)
    bu.run_command = patched
    bu._ldw_patch = True


def kernel(**inputs):
    global last_results
    _maybe_patch_ldw_opt()
    from concourse.bass_utils import run_bass_kernel_spmd

    if "nc" not in _cache:
        _cache["nc"] = _build_program()
    nc = _cache["nc"]

    shared = _host_prep(inputs)
    x1 = np.asarray(inputs["x1"], dtype=np.float32).reshape(B, C, N).astype(BF16)
    x2 = np.asarray(inputs["x2"], dtype=np.float32).reshape(B, C, N).astype(BF16)

    in_maps = []
    for b in range(B):
        m = dict(shared)
        m["x1"] = np.ascontiguousarray(x1[b])
        m["x2"] = np.ascontiguousarray(x2[b])
        in_maps.append(m)

    trace = os.environ.get("BASS_KERNEL_TRACE", "0") == "1"
    res = run_bass_kernel_spmd(
        nc, in_maps, core_ids=list(range(B)), trace=trace
    )
    last_results = res
    out = np.stack([res.results[b]["out"].reshape(C, H, W) for b in range(B)])
    return out.astype(np.float32)
